# revision 22
# baseline (speedup 1.0000x reference)
"""Trainium2 Bass kernel for nn_Block_mamba (SiMBA-style block: Mamba + EinFFT).

8 NeuronCores = 2 batch groups x 4-way shard of d_inner (256 ch/core).
v2 design notes (vector engine is the bottleneck; scans are a fixed
~2.1ns/elem DVE cost, dtype independent):
 - in_proj/conv replicated per core (no AllGather); depthwise conv runs on
   the PE as diagonal matmuls with shifted column windows.
 - selective scan processes both local d-tiles packed in one wide
   [128, 2048] free dim; the j-boundary column of dA is zeroed so one
   hardware scan covers both tiles.
 - dA stays fp32 (scan speed is dtype independent); dB/h/p bf16.
 - p = h*C runs on GpSimd; the s-accumulation is PE matmul-accumulate with
   an identity stationary into PSUM (fp32, exact), D*xm folded in via a
   diagonal-D matmul.
 - residual stream bf16; LN uses PE for stats/broadcast, tensor_scalar 4x.

kernel(**inputs): full unsharded inputs -> full (2, 1024, 512) output.
"""

import numpy as np
import ml_dtypes

DIM = 512
NB = 4
BS = 128
DS = 64
DC = 4
DI = 1024
DTR = 32
BLOCKS = 2
LAM = 0.01
L = 1024

N_CORES = 8
GROUP = 4
DIL = DI // GROUP       # 256
P = 128
NDT = DIL // P          # 2
NCH = DIM // P          # 4
NMT = DI // P           # 8 xm tiles (replicated)
W = 2 * L               # wide free dim (both d-tiles packed)

BF16 = ml_dtypes.bfloat16

_COMPILED = None


def _nt(s):
    return {"name": s, "tag": s}


def _build_program():
    import contextlib
    import concourse.bacc as bacc
    import concourse.mybir as mybir
    import concourse.tile as tile

    F32 = mybir.dt.float32
    BF = mybir.dt.bfloat16
    AF = mybir.ActivationFunctionType
    ALU = mybir.AluOpType

    nc = bacc.Bacc("TRN2", target_bir_lowering=False, debug=False,
                   num_devices=N_CORES)

    _eps = nc.alloc_sbuf_tensor("const-float32-eps", [128, 1], F32)
    nc.gpsimd.memset(_eps.ap(), 1e-5)
    nc.const_aps.aps[(F32, 1e-5)] = _eps.ap()
    nc.all_engine_barrier()

    def din(name, shape, dt=F32):
        return nc.dram_tensor(name, shape, dt, kind="ExternalInput")

    xT_d = din("xT", [DIM, L], BF)
    w_in_d = din("w_in", [DIM, DI + DIL], BF)
    conv_diag_d = din("conv_diag", [NMT * DC, P, P], BF)
    conv_b_d = din("conv_b", [DI, 1])
    w_xp_d = din("w_xp", [DI, DTR + 2 * DS], BF)
    w_dt_d = din("w_dt", [DTR, DIL], BF)
    dt_b_d = din("dt_b", [DIL, 1])
    A_d = din("A", [DIL, DS])
    diag_D_d = din("diag_D", [NDT, P, P], BF)
    w_out_d = din("w_out", [DIL, DIM], BF)
    ln_w_d = din("ln_w", [DIM, 1])
    ln_b_d = din("ln_b", [DIM, 1])
    n2_w_d = din("n2_w", [DIM, 1])
    n2_b_d = din("n2_b", [DIM, 1])
    CdF_d = din("CdF", [L, 256], BF)    # C[:, k1_loc] forward
    SdF_d = din("SdF", [L, 256], BF)
    CdI_d = din("CdI", [256, L], BF)    # C[k1_loc, :] inverse
    SdI_d = din("SdI", [256, L], BF)
    w1r_d = din("w1r", [NB, BS, BS], BF)
    w1i_d = din("w1i", [NB, BS, BS], BF)
    w1in_d = din("w1in", [NB, BS, BS], BF)
    w2r_d = din("w2r", [NB, BS, BS], BF)
    w2i_d = din("w2i", [NB, BS, BS], BF)
    w2in_d = din("w2in", [NB, BS, BS], BF)
    cb1r_d = din("cb1r", [NB, BS, 1])
    cb1i_d = din("cb1i", [NB, BS, 1])
    ssb_d = din("ssb", [NB, 4, BS, 1])
    ident_d = din("ident", [P, P])
    xO_d = nc.dram_tensor("xO", [DIM, L], BF, kind="ExternalOutput")
    xP_d = nc.dram_tensor("xP", [DIM, L], BF, kind="ExternalOutput")

    RG = [[0, 1, 2, 3], [4, 5, 6, 7]]

    with tile.TileContext(nc) as tc:
        stack = contextlib.ExitStack()
        with stack:
            wp = stack.enter_context(tc.tile_pool(name="wp", bufs=1))
            ap = stack.enter_context(tc.tile_pool(name="ap", bufs=1))
            lnp = stack.enter_context(tc.tile_pool(name="lnp", bufs=1))
            dram = stack.enter_context(
                tc.tile_pool(name="dram", bufs=1, space="DRAM"))

            # residual stream (bf16)
            x_res = [ap.tile([P, L], BF, **_nt(f"xres{k}"))
                     for k in range(NCH)]
            for k in range(NCH):
                nc.sync.dma_start(x_res[k][:], xT_d[k * P:(k + 1) * P, :])

            def wtile(src, shape, dt=F32, name=None, tag=None):
                t = wp.tile(shape, dt, name=name, tag=tag)
                nc.sync.dma_start(t[:], src)
                return t

            # critical-path weights first
            ln_w = [wtile(ln_w_d[k * P:(k + 1) * P, :], [P, 1],
                          **_nt(f"lnw{k}")) for k in range(NCH)]
            ln_b = [wtile(ln_b_d[k * P:(k + 1) * P, :], [P, 1],
                          **_nt(f"lnb{k}")) for k in range(NCH)]
            w_in = [wtile(w_in_d[k * P:(k + 1) * P, :], [P, DI + DIL], BF,
                          **_nt(f"w_in{k}")) for k in range(NCH)]
            conv_diag = [wtile(conv_diag_d[i], [P, P], BF,
                               **_nt(f"cvd{i}")) for i in range(NMT * DC)]
            conv_b = [wtile(conv_b_d[m * P:(m + 1) * P, :], [P, 1],
                            **_nt(f"convb{m}")) for m in range(NMT)]
            w_xp = [wtile(w_xp_d[k * P:(k + 1) * P, :], [P, DTR + 2 * DS],
                          BF, **_nt(f"w_xp{k}")) for k in range(NMT)]
            w_dt = wtile(w_dt_d[:], [DTR, DIL], BF, **_nt("w_dt"))
            dt_b = [wtile(dt_b_d[j * P:(j + 1) * P, :], [P, 1],
                          **_nt(f"dtb{j}")) for j in range(NDT)]
            A_t = [wtile(A_d[j * P:(j + 1) * P, :], [P, DS],
                         **_nt(f"A{j}")) for j in range(NDT)]
            diag_D = [wtile(diag_D_d[j], [P, P], BF, **_nt(f"dD{j}"))
                      for j in range(NDT)]
            w_out = [wtile(w_out_d[j * P:(j + 1) * P, :], [P, DIM], BF,
                           **_nt(f"w_out{j}")) for j in range(NDT)]
            n2_w = [wtile(n2_w_d[k * P:(k + 1) * P, :], [P, 1],
                          **_nt(f"n2w{k}")) for k in range(NCH)]
            n2_b = [wtile(n2_b_d[k * P:(k + 1) * P, :], [P, 1],
                          **_nt(f"n2b{k}")) for k in range(NCH)]
            ident = wtile(ident_d[:], [P, P], **_nt("ident"))
            ident_bf = wp.tile([P, P], BF, **_nt("ident_bf"))
            nc.vector.tensor_copy(ident_bf[:], ident[:])
            # einfft weights (needed latest)
            CdF = [wtile(CdF_d[t * P:(t + 1) * P, :], [P, 256], BF,
                         **_nt(f"CdF{t}")) for t in range(8)]
            SdF = [wtile(SdF_d[t * P:(t + 1) * P, :], [P, 256], BF,
                         **_nt(f"SdF{t}")) for t in range(8)]
            CdI = [wtile(CdI_d[c * P:(c + 1) * P, :], [P, L], BF,
                         **_nt(f"CdI{c}")) for c in range(2)]
            SdI = [wtile(SdI_d[c * P:(c + 1) * P, :], [P, L], BF,
                         **_nt(f"SdI{c}")) for c in range(2)]
            w1r = [wtile(w1r_d[b], [BS, BS], BF, **_nt(f"w1r{b}"))
                   for b in range(NB)]
            w1i = [wtile(w1i_d[b], [BS, BS], BF, **_nt(f"w1i{b}"))
                   for b in range(NB)]
            w1in = [wtile(w1in_d[b], [BS, BS], BF, **_nt(f"w1in{b}"))
                    for b in range(NB)]
            w2r = [wtile(w2r_d[b], [BS, BS], BF, **_nt(f"w2r{b}"))
                   for b in range(NB)]
            w2i = [wtile(w2i_d[b], [BS, BS], BF, **_nt(f"w2i{b}"))
                   for b in range(NB)]
            w2in = [wtile(w2in_d[b], [BS, BS], BF, **_nt(f"w2in{b}"))
                    for b in range(NB)]
            cb1r = [wtile(cb1r_d[b], [BS, 1], **_nt(f"cb1r{b}"))
                    for b in range(NB)]
            cb1i = [wtile(cb1i_d[b], [BS, 1], **_nt(f"cb1i{b}"))
                    for b in range(NB)]
            ssb = [[wtile(ssb_d[b, jj], [BS, 1], **_nt(f"ssb{b}_{jj}"))
                    for jj in range(4)] for b in range(NB)]

            ones_k1 = wp.tile([1, P], BF, **_nt("ones_k1"))
            nc.vector.memset(ones_k1[:], 1.0)
            ones_m1 = wp.tile([P, 1], BF, **_nt("ones_m1"))
            nc.vector.memset(ones_m1[:], 1.0)

            # ----------------------------------------------------------
            def layer_norm(w_aps, b_aps, pool, out_tag):
                """bf16 LN over partition dim (d) of x_res; returns bf16."""
                with tc.tile_pool(name="psln", bufs=1, space="PSUM") as psl:
                    pm = psl.tile([1, L], F32, **_nt("ln_pm"))
                    for k in range(NCH):
                        for h in range(2):
                            hs = slice(h * 512, (h + 1) * 512)
                            nc.tensor.matmul(pm[:, hs], ones_m1[:],
                                             x_res[k][:, hs],
                                             start=(k == 0),
                                             stop=(k == NCH - 1))
                    psq = psl.tile([1, L], F32, **_nt("ln_psq"))
                    for k in range(NCH):
                        x2 = lnp.tile([P, L], BF, **_nt("ln_x2"), bufs=2)
                        nc.scalar.activation(x2[:], x_res[k][:], AF.Square)
                        for h in range(2):
                            hs = slice(h * 512, (h + 1) * 512)
                            nc.tensor.matmul(psq[:, hs], ones_m1[:],
                                             x2[:, hs], start=(k == 0),
                                             stop=(k == NCH - 1))
                    nm = lnp.tile([1, L], BF, **_nt("ln_nm"))
                    nc.scalar.mul(nm[:], pm[:], -1.0 / DIM)
                    msq = lnp.tile([1, L], F32, **_nt("ln_msq"))
                    nc.scalar.activation(msq[:], nm[:], AF.Square)
                    ch = lnp.tile([1, L], F32, **_nt("ln_ch"))
                    nc.vector.scalar_tensor_tensor(
                        ch[:], psq[:], 1.0 / DIM, msq[:],
                        ALU.mult, ALU.subtract)
                    inv = lnp.tile([1, L], BF, **_nt("ln_inv"))
                    nc.scalar.activation(inv[:], ch[:],
                                         AF.Abs_reciprocal_sqrt, bias=1e-5)
                    nm_ps = psl.tile([P, L], F32, **_nt("ln_nmps"))
                    iv_ps = psl.tile([P, L], F32, **_nt("ln_ivps"))
                    for h in range(2):
                        hs = slice(h * 512, (h + 1) * 512)
                        nc.tensor.matmul(nm_ps[:, hs], ones_k1[:],
                                         nm[:, hs], start=True, stop=True)
                        nc.tensor.matmul(iv_ps[:, hs], ones_k1[:],
                                         inv[:, hs], start=True, stop=True)
                    nm_bc = lnp.tile([P, L], BF, **_nt("ln_nmbc"))
                    nc.vector.tensor_copy(nm_bc[:], nm_ps[:])
                    iv_bc = lnp.tile([P, L], BF, **_nt("ln_ivbc"))
                    nc.vector.tensor_copy(iv_bc[:], iv_ps[:])
                    outs = []
                    for k in range(NCH):
                        t1 = lnp.tile([P, L], BF, **_nt("ln_t1"), bufs=3)
                        nc.vector.tensor_tensor(t1[:], x_res[k][:],
                                                nm_bc[:], ALU.add)
                        t2 = lnp.tile([P, L], BF, **_nt("ln_t2"), bufs=3)
                        nc.vector.tensor_tensor(t2[:], t1[:], iv_bc[:],
                                                ALU.mult)
                        o = pool.tile([P, L], BF, **_nt(f"{out_tag}{k}"))
                        nc.vector.tensor_scalar(o[:], t2[:], w_aps[k][:],
                                                b_aps[k][:], ALU.mult,
                                                ALU.add)
                        outs.append(o)
                    return outs

            # ----------------------------------------------------------
            def mamba_block(blk):
                with tc.tile_pool(name="mb", bufs=1) as mb:
                    ar2_in = [dram.tile([DIM // 2, L], BF, **_nt(f"ar2i{c}"))
                              for c in range(2)]
                    ar2_out = [dram.tile([DIM // 2, L], BF,
                                         **_nt(f"ar2o{c}")) for c in range(2)]
                    projbd = dram.tile([2 * DS, L], BF, **_nt("projbd"))

                    xmc_wide = mb.tile([P, W], BF, **_nt("xmc_wide"))
                    szs_wide = mb.tile([P, W], BF, **_nt("szs_wide"))
                    dt_wide = mb.tile([P, W], F32, **_nt("dt_wide"))
                    du_wide = mb.tile([P, W], BF, **_nt("du_wide"))
                    proj_dt = mb.tile([DTR, L], BF, **_nt("proj_dt"))
                    y2 = [mb.tile([P, L], BF, **_nt(f"y2_{j}"))
                          for j in range(NDT)]

                    with tc.tile_pool(name="mpre", bufs=1) as mpre:
                        xn = layer_norm(ln_w, ln_b, mpre, "xn")
                        # --- in_proj (all 8 xm tiles + own 2 z) pipelined
                        # with the PE depthwise conv + x_proj accumulation.
                        LP = L + DC - 1
                        xm_sb = [mpre.tile([P, LP], BF, **_nt(f"xmsb{m}"))
                                 for m in range(NMT)]
                        for m in range(NMT):
                            nc.gpsimd.memset(xm_sb[m][:, 0:DC - 1], 0.0)
                        with tc.tile_pool(name="psA", bufs=1,
                                          space="PSUM") as psA, \
                             tc.tile_pool(name="psC", bufs=1,
                                          space="PSUM") as psC, \
                             tc.tile_pool(name="psP", bufs=1,
                                          space="PSUM") as psP:
                            pp1 = psP.tile([P, L], F32, **_nt("pp1"))
                            pp2 = psP.tile([32, L], F32, **_nt("pp2"))
                            xmc = [None] * NMT

                            def emit_inproj(mt):
                                pxz = psA.tile([P, L], F32, **_nt("pxz"))
                                for k in range(NCH):
                                    lhs = w_in[k][:, mt * P:(mt + 1) * P]
                                    for h in range(2):
                                        hs = slice(h * 512, (h + 1) * 512)
                                        nc.tensor.matmul(
                                            pxz[:, hs], lhs, xn[k][:, hs],
                                            start=(k == 0),
                                            stop=(k == NCH - 1))
                                if mt < NMT:
                                    nc.vector.tensor_copy(
                                        xm_sb[mt][:, DC - 1:LP], pxz[:])
                                else:
                                    j = mt - NMT
                                    nc.scalar.activation(
                                        szs_wide[:, j * L:(j + 1) * L],
                                        pxz[:], AF.Silu)

                            def emit_conv(mt):
                                psc = psC.tile([P, L], F32, **_nt("psc"))
                                for q in range(DC):
                                    dg = conv_diag[mt * DC + q]
                                    for h in range(2):
                                        hs = slice(h * 512, (h + 1) * 512)
                                        nc.tensor.matmul(
                                            psc[:, hs], dg[:],
                                            xm_sb[mt][:, q + h * 512:
                                                       q + (h + 1) * 512],
                                            start=(q == 0),
                                            stop=(q == DC - 1))
                                # own-first channel order: own d-tiles are
                                # mt 0..NDT-1.
                                if mt < NDT:
                                    nc.scalar.activation(
                                        xmc_wide[:, mt * L:(mt + 1) * L],
                                        psc[:], AF.Silu,
                                        bias=conv_b[mt][:])
                                    xmc[mt] = (xmc_wide,
                                               slice(mt * L, (mt + 1) * L))
                                else:
                                    t = mpre.tile([P, L], BF,
                                                  **_nt("xmct"), bufs=4)
                                    nc.scalar.activation(t[:], psc[:],
                                                         AF.Silu,
                                                         bias=conv_b[mt][:])
                                    xmc[mt] = (t, slice(0, L))
                                src, sl = xmc[mt]
                                for h in range(2):
                                    hs = slice(h * 512, (h + 1) * 512)
                                    rhs = src[:, sl.start + h * 512:
                                              sl.start + (h + 1) * 512]
                                    nc.tensor.matmul(
                                        pp1[:, hs], w_xp[mt][:, 0:P], rhs,
                                        start=(mt == 0),
                                        stop=(mt == NMT - 1))
                                    nc.tensor.matmul(
                                        pp2[:, hs], w_xp[mt][:, P:160], rhs,
                                        start=(mt == 0),
                                        stop=(mt == NMT - 1))

                            for mt in range(NMT + NDT):
                                emit_inproj(mt)
                                if 1 <= mt <= NMT:
                                    emit_conv(mt - 1)
                            # extract dt-proj input + B/C rows
                            nc.vector.tensor_copy(proj_dt[:],
                                                  pp1[0:DTR, :])
                            pjA = mpre.tile([P, L], BF, **_nt("pjA"))
                            nc.vector.tensor_copy(pjA[:], pp1[:])
                            pjB = mpre.tile([32, L], BF, **_nt("pjB"))
                            nc.vector.tensor_copy(pjB[:], pp2[:])
                            nc.sync.dma_start(projbd[0:96, :],
                                              pjA[DTR:P, :])
                            nc.sync.dma_start(projbd[96:128, :], pjB[:])
                        # --- dt_proj + softplus into dt_wide halves ---
                        with tc.tile_pool(name="psD", bufs=2,
                                          space="PSUM") as psD:
                            for j in range(NDT):
                                pdt = psD.tile([P, L], F32, **_nt("pdt"))
                                for h in range(2):
                                    hs = slice(h * 512, (h + 1) * 512)
                                    nc.tensor.matmul(
                                        pdt[:, hs],
                                        w_dt[:, j * P:(j + 1) * P],
                                        proj_dt[:, hs], start=True,
                                        stop=True)
                                dtj = dt_wide[:, j * L:(j + 1) * L]
                                nc.scalar.activation(dtj, pdt[:], AF.Exp,
                                                     bias=dt_b[j][:])
                                nc.scalar.activation(dtj, dtj, AF.Ln,
                                                     bias=1.0)
                        nc.vector.tensor_tensor(du_wide[:], dt_wide[:],
                                                xmc_wide[:], ALU.mult)

                    # ------------- scan loop --------------------------
                    with tc.tile_pool(name="msc", bufs=1) as msc, \
                         tc.tile_pool(name="psY", bufs=1,
                                      space="PSUM") as psY:
                        psum_y = [psY.tile([P, L], F32, **_nt(f"py{j}"))
                                  for j in range(NDT)]
                        # init: psum_y[j] = diag(D_j) @ xmc_j
                        for j in range(NDT):
                            for h in range(2):
                                hs = slice(h * 512, (h + 1) * 512)
                                nc.tensor.matmul(
                                    psum_y[j][:, hs], diag_D[j][:],
                                    xmc_wide[:, j * L + h * 512:
                                             j * L + (h + 1) * 512],
                                    start=True, stop=False)
                        for s in range(DS):
                            # one broadcast per s, shared by both d-tiles
                            bB = msc.tile([P, L], BF, **_nt("bB"), bufs=6)
                            nc.sync.dma_start(
                                bB[:],
                                projbd[s:s + 1, :].to_broadcast((P, L)))
                            bC = msc.tile([P, L], BF, **_nt("bC"), bufs=6)
                            nc.sync.dma_start(
                                bC[:],
                                projbd[DS + s:DS + s + 1,
                                       :].to_broadcast((P, L)))
                            last = (s == DS - 1)
                            for j in range(NDT):
                                js = slice(j * L, (j + 1) * L)
                                dA = msc.tile([P, L], F32, **_nt("dA"),
                                              bufs=4)
                                nc.scalar.activation(
                                    dA[:], dt_wide[:, js], AF.Exp,
                                    scale=A_t[j][:, s:s + 1])
                                dB = msc.tile([P, L], BF, **_nt("dB"),
                                              bufs=3)
                                nc.vector.tensor_tensor(
                                    dB[:], du_wide[:, js], bB[:], ALU.mult)
                                h_t = msc.tile([P, L], BF, **_nt("h"),
                                               bufs=3)
                                nc.vector.tensor_tensor_scan(
                                    h_t[:], dA[:], dB[:], 0.0,
                                    ALU.mult, ALU.add)
                                p = msc.tile([P, L], BF, **_nt("p"),
                                             bufs=3)
                                nc.vector.tensor_tensor(p[:], h_t[:],
                                                        bC[:], ALU.mult)
                                for h in range(2):
                                    hs = slice(h * 512, (h + 1) * 512)
                                    nc.tensor.matmul(
                                        psum_y[j][:, hs], ident_bf[:],
                                        p[:, hs], start=False, stop=last)
                        # --- gate + out_proj ---
                        for j in range(NDT):
                            nc.vector.tensor_tensor(
                                y2[j][:], psum_y[j][:],
                                szs_wide[:, j * L:(j + 1) * L], ALU.mult)
                    with tc.tile_pool(name="mpost", bufs=1) as mpost, \
                         tc.tile_pool(name="psO", bufs=2,
                                      space="PSUM") as psO:
                        for mt in range(NCH):
                            po = psO.tile([P, L], F32, **_nt("pout"))
                            for h in range(2):
                                hs = slice(h * 512, (h + 1) * 512)
                                for j in range(NDT):
                                    nc.tensor.matmul(
                                        po[:, hs],
                                        w_out[j][:, mt * P:(mt + 1) * P],
                                        y2[j][:, hs], start=(j == 0),
                                        stop=(j == NDT - 1))
                            osb = mpost.tile([P, L], BF, **_nt("ar2sb"),
                                             bufs=2)
                            nc.vector.tensor_copy(osb[:], po[:])
                            c, rr = divmod(mt, 2)
                            nc.sync.dma_start(
                                ar2_in[c][rr * P:(rr + 1) * P, :], osb[:])
                            if rr == 1:
                                nc.gpsimd.collective_compute(
                                    "AllReduce", ALU.add,
                                    replica_groups=RG,
                                    ins=[ar2_in[c].opt()],
                                    outs=[ar2_out[c].opt()])
                        for k in range(NCH):
                            c, rr = divmod(k, 2)
                            mo = mpost.tile([P, L], BF, **_nt("mo"),
                                            bufs=2)
                            nc.sync.dma_start(
                                mo[:], ar2_out[c][rr * P:(rr + 1) * P, :])
                            nc.vector.tensor_tensor(x_res[k][:],
                                                    x_res[k][:], mo[:],
                                                    ALU.add)

            # ----------------------------------------------------------
            def bfly(pool, pl, tagp, Wb=L):
                R, I = pl[:4], pl[4:]
                t_ = {}
                for nm, (a, b, op) in {
                    "SR": (R[0], R[2], ALU.add),
                    "DR": (R[0], R[2], ALU.subtract),
                    "SR2": (R[1], R[3], ALU.add),
                    "DR2": (R[1], R[3], ALU.subtract),
                    "SI": (I[0], I[2], ALU.add),
                    "DI": (I[0], I[2], ALU.subtract),
                    "SI2": (I[1], I[3], ALU.add),
                    "DI2": (I[1], I[3], ALU.subtract),
                }.items():
                    tt = pool.tile([P, Wb], BF, **_nt(f"{tagp}t_{nm}"))
                    nc.vector.tensor_tensor(tt[:], a[:], b[:], op)
                    t_[nm] = tt
                spec = [("SR", "SR2", ALU.add), ("DR", "DI2", ALU.add),
                        ("SR", "SR2", ALU.subtract),
                        ("DR", "DI2", ALU.subtract),
                        ("SI", "SI2", ALU.add), ("DI", "DR2", ALU.subtract),
                        ("SI", "SI2", ALU.subtract), ("DI", "DR2", ALU.add)]
                out = []
                for i, (a, b, op) in enumerate(spec):
                    o = pool.tile([P, Wb], BF, **_nt(f"{tagp}o{i}"))
                    nc.vector.tensor_tensor(o[:], t_[a][:], t_[b][:], op)
                    out.append(o)
                return out[:4], out[4:]

            def einfft_block(last=False):
                KL = 256          # local k1 width
                with tc.tile_pool(name="ef", bufs=1) as ef:
                    ar3_in = [dram.tile([DIM // 2, L], BF, **_nt(f"ar3i{c}"))
                              for c in range(2)]
                    ar3_out = [dram.tile([DIM // 2, L], BF,
                                         **_nt(f"ar3o{c}")) for c in range(2)]
                    Xre = [ef.tile([P, KL], BF, **_nt(f"Xre{k}"))
                           for k in range(NCH)]
                    Xim = [ef.tile([P, KL], BF, **_nt(f"Xim{k}"))
                           for k in range(NCH)]
                    with tc.tile_pool(name="efa", bufs=1) as efa:
                        xn2 = layer_norm(n2_w, n2_b, efa, "xn2")
                        xnT = [efa.tile([P, DIM], BF, **_nt(f"xnT{t}"))
                               for t in range(8)]
                        with tc.tile_pool(name="psF", bufs=1,
                                          space="PSUM") as psF:
                            for t in range(8):
                                for k in range(NCH):
                                    pt = psF.tile([P, P], BF, **_nt("ptp"),
                                                  bufs=2)
                                    nc.tensor.transpose(
                                        pt[:], xn2[k][:, t * P:(t + 1) * P],
                                        ident_bf[:])
                                    nc.vector.tensor_copy(
                                        xnT[t][:, k * P:(k + 1) * P], pt[:])
                            for k in range(NCH):
                                pre = psF.tile([P, KL], F32, **_nt("pfr"),
                                               bufs=2)
                                pim = psF.tile([P, KL], F32, **_nt("pfi"),
                                               bufs=2)
                                for t in range(8):
                                    lhs = xnT[t][:, k * P:(k + 1) * P]
                                    nc.tensor.matmul(pre[:], lhs, CdF[t][:],
                                                     start=(t == 0),
                                                     stop=(t == 7))
                                    nc.tensor.matmul(pim[:], lhs, SdF[t][:],
                                                     start=(t == 0),
                                                     stop=(t == 7))
                                nc.vector.tensor_copy(Xre[k][:], pre[:])
                                nc.vector.tensor_scalar_mul(Xim[k][:],
                                                            pim[:], -1.0)

                    Xf_re, Xf_im = bfly(ef, Xre + Xim, "ff", KL)

                    r1 = [ef.tile([P, KL], BF, **_nt(f"r1_{b}"))
                          for b in range(NB)]
                    i1 = [ef.tile([P, KL], BF, **_nt(f"i1_{b}"))
                          for b in range(NB)]
                    with tc.tile_pool(name="psL1", bufs=2,
                                      space="PSUM") as psL1:
                        for b in range(NB):
                            pr = psL1.tile([P, KL], F32, **_nt("pl1r"))
                            nc.tensor.matmul(pr[:], w1r[b][:], Xf_re[b][:],
                                             start=True, stop=False)
                            nc.tensor.matmul(pr[:], w1in[b][:], Xf_im[b][:],
                                             start=False, stop=True)
                            nc.scalar.activation(r1[b][:], pr[:], AF.Relu,
                                                 bias=cb1r[b][:])
                            pi = psL1.tile([P, KL], F32, **_nt("pl1i"))
                            nc.tensor.matmul(pi[:], w1i[b][:], Xf_re[b][:],
                                             start=True, stop=False)
                            nc.tensor.matmul(pi[:], w1r[b][:], Xf_im[b][:],
                                             start=False, stop=True)
                            nc.scalar.activation(i1[b][:], pi[:], AF.Relu,
                                                 bias=cb1i[b][:])

                    zre = [None] * NB
                    zimN = [None] * NB
                    with tc.tile_pool(name="psL2", bufs=2,
                                      space="PSUM") as psL2:
                        for b in range(NB):
                            pzr = psL2.tile([P, KL], F32, **_nt("pl2r"))
                            nc.tensor.matmul(pzr[:], w2r[b][:], r1[b][:],
                                             start=True, stop=False)
                            nc.tensor.matmul(pzr[:], w2in[b][:], i1[b][:],
                                             start=False, stop=True)
                            a1 = ef.tile([P, KL], BF, **_nt("ss"), bufs=4)
                            nc.scalar.activation(a1[:], pzr[:], AF.Relu,
                                                 scale=0.5, bias=ssb[b][0][:])
                            a2 = ef.tile([P, KL], BF, **_nt("ss"), bufs=4)
                            nc.scalar.activation(a2[:], pzr[:], AF.Relu,
                                                 scale=-0.5,
                                                 bias=ssb[b][1][:])
                            zr = ef.tile([P, KL], BF, name=f"zre{b}",
                                         tag=f"Xre{b}")
                            nc.vector.tensor_tensor(zr[:], a1[:], a2[:],
                                                    ALU.subtract)
                            zre[b] = zr
                            pzi = psL2.tile([P, KL], F32, **_nt("pl2i"))
                            nc.tensor.matmul(pzi[:], w2i[b][:], r1[b][:],
                                             start=True, stop=False)
                            nc.tensor.matmul(pzi[:], w2r[b][:], i1[b][:],
                                             start=False, stop=True)
                            b1 = ef.tile([P, KL], BF, **_nt("ss"), bufs=4)
                            nc.scalar.activation(b1[:], pzi[:], AF.Relu,
                                                 scale=0.5, bias=ssb[b][2][:])
                            b2 = ef.tile([P, KL], BF, **_nt("ss"), bufs=4)
                            nc.scalar.activation(b2[:], pzi[:], AF.Relu,
                                                 scale=-0.5,
                                                 bias=ssb[b][3][:])
                            zi = ef.tile([P, KL], BF, name=f"zimN{b}",
                                         tag=f"Xim{b}")
                            nc.vector.tensor_tensor(zi[:], b2[:], b1[:],
                                                    ALU.subtract)
                            zimN[b] = zi

                    zz_re, zz_iN = bfly(ef, zre + zimN, "ff", KL)

                    with tc.tile_pool(name="psI", bufs=2,
                                      space="PSUM") as psI:
                        for b in range(NB):
                            zTr = ef.tile([P, KL], BF, **_nt("zzTr"),
                                          bufs=2)
                            zTi = ef.tile([P, KL], BF, **_nt("zzTi"),
                                          bufs=2)
                            for c in range(2):
                                pt = psI.tile([P, P], BF, **_nt("ptp2"))
                                nc.tensor.transpose(
                                    pt[:], zz_re[b][:, c * P:(c + 1) * P],
                                    ident_bf[:])
                                nc.vector.tensor_copy(
                                    zTr[:, c * P:(c + 1) * P], pt[:])
                                pt2 = psI.tile([P, P], BF, **_nt("ptp3"))
                                nc.tensor.transpose(
                                    pt2[:], zz_iN[b][:, c * P:(c + 1) * P],
                                    ident_bf[:])
                                nc.vector.tensor_copy(
                                    zTi[:, c * P:(c + 1) * P], pt2[:])
                            for h in range(2):
                                hs = slice(h * 512, (h + 1) * 512)
                                pout = psI.tile([P, 512], F32,
                                                **_nt("pidft"))
                                for c in range(2):
                                    nc.tensor.matmul(
                                        pout[:], zTr[:, c * P:(c + 1) * P],
                                        CdI[c][:, hs], start=(c == 0),
                                        stop=False)
                                    nc.tensor.matmul(
                                        pout[:], zTi[:, c * P:(c + 1) * P],
                                        SdI[c][:, hs], start=False,
                                        stop=(c == 1))
                                ob = ef.tile([P, 512], BF, **_nt("eob"),
                                             bufs=3)
                                nc.vector.tensor_copy(ob[:], pout[:])
                                if last:
                                    nc.sync.dma_start(
                                        xP_d[b * P:(b + 1) * P, hs], ob[:])
                                else:
                                    c, rr = divmod(b, 2)
                                    nc.sync.dma_start(
                                        ar3_in[c][rr * P:(rr + 1) * P, hs],
                                        ob[:])
                            if not last and b % 2 == 1:
                                c = b // 2
                                nc.gpsimd.collective_compute(
                                    "AllReduce", ALU.add,
                                    replica_groups=RG,
                                    ins=[ar3_in[c].opt()],
                                    outs=[ar3_out[c].opt()])

                    if not last:
                        for k in range(NCH):
                            c, rr = divmod(k, 2)
                            eo = ef.tile([P, L], BF, **_nt("eo"), bufs=2)
                            nc.sync.dma_start(
                                eo[:], ar3_out[c][rr * P:(rr + 1) * P, :])
                            nc.vector.tensor_tensor(x_res[k][:],
                                                    x_res[k][:], eo[:],
                                                    ALU.add)

            for blk in range(BLOCKS):
                mamba_block(blk)
                if blk == BLOCKS - 1:
                    for k in range(NCH):
                        nc.sync.dma_start(xO_d[k * P:(k + 1) * P, :],
                                          x_res[k][:])
                einfft_block(last=(blk == BLOCKS - 1))

    nc.compile()
    return nc


# --------------------------------------------------------------------------

def _make_inmaps(inputs):
    f32 = np.float32
    x = np.asarray(inputs["x"], f32)
    in_proj_w = np.asarray(inputs["in_proj_w"], f32)
    conv_w = np.asarray(inputs["conv_w"], f32)
    conv_b = np.asarray(inputs["conv_b"], f32)
    x_proj_w = np.asarray(inputs["x_proj_w"], f32)
    dt_proj_w = np.asarray(inputs["dt_proj_w"], f32)
    dt_proj_b = np.asarray(inputs["dt_proj_b"], f32)
    A_log = np.asarray(inputs["A_log"], f32)
    Dvec = np.asarray(inputs["D"], f32)
    out_proj_w = np.asarray(inputs["out_proj_w"], f32)
    ln_w = np.asarray(inputs["ln_w"], f32)
    ln_b = np.asarray(inputs["ln_b"], f32)
    n2_w = np.asarray(inputs["norm2_w"], f32)
    n2_b = np.asarray(inputs["norm2_b"], f32)
    cw1 = np.asarray(inputs["cw1"], f32)
    cw2 = np.asarray(inputs["cw2"], f32)
    cb1 = np.asarray(inputs["cb1"], f32)
    cb2 = np.asarray(inputs["cb2"], f32)

    n = np.arange(L, dtype=np.float64)
    ang = 2.0 * np.pi * np.outer(n, n) / L
    Cdft = (np.cos(ang) / np.sqrt(L)).astype(BF16)
    Sdft = (np.sin(ang) / np.sqrt(L)).astype(BF16)
    CdF = [np.ascontiguousarray(Cdft[:, r * 256:(r + 1) * 256])
           for r in range(GROUP)]
    SdF = [np.ascontiguousarray(Sdft[:, r * 256:(r + 1) * 256])
           for r in range(GROUP)]
    CdI = [np.ascontiguousarray(Cdft[r * 256:(r + 1) * 256, :])
           for r in range(GROUP)]
    SdI = [np.ascontiguousarray(Sdft[r * 256:(r + 1) * 256, :])
           for r in range(GROUP)]

    ssb = np.stack([
        (cb2[0] - LAM) / 2, (-cb2[0] - LAM) / 2,
        (cb2[1] - LAM) / 2, (-cb2[1] - LAM) / 2,
    ], axis=1)[:, :, :, None]

    in_maps = []
    for core in range(N_CORES):
        g, r = divmod(core, GROUP)
        lo, hi = r * DIL, (r + 1) * DIL
        # per-core channel order: own d_inner slice first, then the rest
        perm = np.r_[lo:hi, 0:lo, hi:DI]
        conv_wp = conv_w[perm, 0, :]
        conv_diag = np.zeros((NMT * DC, P, P), f32)
        for mt in range(NMT):
            for q in range(DC):
                np.fill_diagonal(conv_diag[mt * DC + q],
                                 conv_wp[mt * P:(mt + 1) * P, q])
        diag_D = np.zeros((NDT, P, P), f32)
        for j in range(NDT):
            np.fill_diagonal(diag_D[j], Dvec[lo + j * P: lo + (j + 1) * P])
        m = {
            "xT": np.ascontiguousarray(x[g].T).astype(BF16),
            "w_in": np.ascontiguousarray(
                np.concatenate([in_proj_w[perm],
                                in_proj_w[DI + lo:DI + hi]], 0).T
            ).astype(BF16),
            "conv_diag": conv_diag.astype(BF16),
            "conv_b": np.ascontiguousarray(conv_b[perm][:, None]),
            "w_xp": np.ascontiguousarray(x_proj_w[:, perm].T).astype(BF16),
            "w_dt": np.ascontiguousarray(dt_proj_w[lo:hi].T).astype(BF16),
            "dt_b": np.ascontiguousarray(dt_proj_b[lo:hi][:, None]),
            "A": np.ascontiguousarray(-np.exp(A_log[lo:hi])),
            "diag_D": diag_D.astype(BF16),
            "w_out": np.ascontiguousarray(
                out_proj_w[:, lo:hi].T).astype(BF16),
            "ln_w": np.ascontiguousarray(ln_w[:, None]),
            "ln_b": np.ascontiguousarray(ln_b[:, None]),
            "n2_w": np.ascontiguousarray(n2_w[:, None]),
            "n2_b": np.ascontiguousarray(n2_b[:, None]),
            "CdF": CdF[r], "SdF": SdF[r],
            "CdI": CdI[r], "SdI": SdI[r],
            "w1r": (0.5 * cw1[0]).astype(BF16),
            "w1i": (0.5 * cw1[1]).astype(BF16),
            "w1in": (-0.5 * cw1[1]).astype(BF16),
            "w2r": cw2[0].astype(BF16),
            "w2i": cw2[1].astype(BF16),
            "w2in": (-cw2[1]).astype(BF16),
            "cb1r": np.ascontiguousarray(cb1[0][:, :, None]),
            "cb1i": np.ascontiguousarray(cb1[1][:, :, None]),
            "ssb": np.ascontiguousarray(ssb, f32),
            "ident": np.eye(P, dtype=f32),
        }
        in_maps.append(m)
    return in_maps


def kernel(**inputs):
    global _COMPILED
    from concourse.bass_utils import run_bass_kernel_spmd
    if _COMPILED is None:
        _COMPILED = _build_program()
    in_maps = _make_inmaps(inputs)
    res = run_bass_kernel_spmd(_COMPILED, in_maps,
                               core_ids=list(range(N_CORES)))
    outs = []
    for g in range(2):
        x = res.results[g * GROUP]["xO"].astype(np.float32)
        for r in range(GROUP):
            x = x + res.results[g * GROUP + r]["xP"].astype(np.float32)
        outs.append(x.T)
    return np.ascontiguousarray(np.stack(outs).astype(np.float32))


# revision 26
# speedup vs baseline: 1.0162x; 1.0162x over previous
"""Trainium2 Bass kernel for nn_Block_mamba (SiMBA-style block: Mamba + EinFFT).

8 NeuronCores = 2 batch groups x 4-way shard of d_inner (256 ch/core).
v2 design notes (vector engine is the bottleneck; scans are a fixed
~2.1ns/elem DVE cost, dtype independent):
 - in_proj/conv replicated per core (no AllGather); depthwise conv runs on
   the PE as diagonal matmuls with shifted column windows.
 - selective scan processes both local d-tiles packed in one wide
   [128, 2048] free dim; the j-boundary column of dA is zeroed so one
   hardware scan covers both tiles.
 - dA stays fp32 (scan speed is dtype independent); dB/h/p bf16.
 - p = h*C runs on GpSimd; the s-accumulation is PE matmul-accumulate with
   an identity stationary into PSUM (fp32, exact), D*xm folded in via a
   diagonal-D matmul.
 - residual stream bf16; LN uses PE for stats/broadcast, tensor_scalar 4x.

kernel(**inputs): full unsharded inputs -> full (2, 1024, 512) output.
"""

import numpy as np
import ml_dtypes

DIM = 512
NB = 4
BS = 128
DS = 64
DC = 4
DI = 1024
DTR = 32
BLOCKS = 2
LAM = 0.01
L = 1024

N_CORES = 8
GROUP = 4
DIL = DI // GROUP       # 256
P = 128
NDT = DIL // P          # 2
NCH = DIM // P          # 4
NMT = DI // P           # 8 xm tiles (replicated)
W = 2 * L               # wide free dim (both d-tiles packed)

BF16 = ml_dtypes.bfloat16

_COMPILED = None


def _nt(s):
    return {"name": s, "tag": s}


def _build_program():
    import contextlib
    import concourse.bacc as bacc
    import concourse.mybir as mybir
    import concourse.tile as tile

    F32 = mybir.dt.float32
    BF = mybir.dt.bfloat16
    AF = mybir.ActivationFunctionType
    ALU = mybir.AluOpType

    nc = bacc.Bacc("TRN2", target_bir_lowering=False, debug=False,
                   num_devices=N_CORES)

    _eps = nc.alloc_sbuf_tensor("const-float32-eps", [128, 1], F32)
    nc.gpsimd.memset(_eps.ap(), 1e-5)
    nc.const_aps.aps[(F32, 1e-5)] = _eps.ap()
    nc.all_engine_barrier()

    def din(name, shape, dt=F32):
        return nc.dram_tensor(name, shape, dt, kind="ExternalInput")

    xT_d = din("xT", [DIM, L], BF)
    w_in_d = din("w_in", [DIM, DI + DIL], BF)
    conv_diag_d = din("conv_diag", [NMT * DC, P, P], BF)
    conv_b_d = din("conv_b", [DI, 1])
    w_xp_d = din("w_xp", [DI, DTR + 2 * DS], BF)
    w_dt_d = din("w_dt", [DTR, DIL], BF)
    dt_b_d = din("dt_b", [DIL, 1])
    A_d = din("A", [DIL, DS])
    diag_D_d = din("diag_D", [NDT, P, P], BF)
    w_out_d = din("w_out", [DIL, DIM], BF)
    ln_w_d = din("ln_w", [DIM, 1])
    ln_b_d = din("ln_b", [DIM, 1])
    n2_w_d = din("n2_w", [DIM, 1])
    n2_b_d = din("n2_b", [DIM, 1])
    CdF_d = din("CdF", [L, 256], BF)    # C[:, k1_loc] forward
    SdF_d = din("SdF", [L, 256], BF)
    CdI_d = din("CdI", [256, L], BF)    # C[k1_loc, :] inverse
    SdI_d = din("SdI", [256, L], BF)
    w1r_d = din("w1r", [NB, BS, BS], BF)
    w1i_d = din("w1i", [NB, BS, BS], BF)
    w1in_d = din("w1in", [NB, BS, BS], BF)
    w2r_d = din("w2r", [NB, BS, BS], BF)
    w2i_d = din("w2i", [NB, BS, BS], BF)
    w2in_d = din("w2in", [NB, BS, BS], BF)
    cb1r_d = din("cb1r", [NB, BS, 1])
    cb1i_d = din("cb1i", [NB, BS, 1])
    ssb_d = din("ssb", [NB, 4, BS, 1])
    ident_d = din("ident", [P, P])
    xO_d = nc.dram_tensor("xO", [DIM, L], BF, kind="ExternalOutput")
    xP_d = nc.dram_tensor("xP", [DIM, L], BF, kind="ExternalOutput")

    RG = [[0, 1, 2, 3], [4, 5, 6, 7]]

    with tile.TileContext(nc) as tc:
        stack = contextlib.ExitStack()
        with stack:
            wp = stack.enter_context(tc.tile_pool(name="wp", bufs=1))
            ap = stack.enter_context(tc.tile_pool(name="ap", bufs=1))
            lnp = stack.enter_context(tc.tile_pool(name="lnp", bufs=1))
            dram = stack.enter_context(
                tc.tile_pool(name="dram", bufs=1, space="DRAM"))

            # residual stream (bf16)
            x_res = [ap.tile([P, L], BF, **_nt(f"xres{k}"))
                     for k in range(NCH)]
            for k in range(NCH):
                nc.sync.dma_start(x_res[k][:], xT_d[k * P:(k + 1) * P, :])

            def wtile(src, shape, dt=F32, name=None, tag=None):
                t = wp.tile(shape, dt, name=name, tag=tag)
                nc.sync.dma_start(t[:], src)
                return t

            # critical-path weights first
            ln_w = [wtile(ln_w_d[k * P:(k + 1) * P, :], [P, 1],
                          **_nt(f"lnw{k}")) for k in range(NCH)]
            ln_b = [wtile(ln_b_d[k * P:(k + 1) * P, :], [P, 1],
                          **_nt(f"lnb{k}")) for k in range(NCH)]
            w_in = [wtile(w_in_d[k * P:(k + 1) * P, :], [P, DI + DIL], BF,
                          **_nt(f"w_in{k}")) for k in range(NCH)]
            conv_diag = [wtile(conv_diag_d[i], [P, P], BF,
                               **_nt(f"cvd{i}")) for i in range(NMT * DC)]
            conv_b = [wtile(conv_b_d[m * P:(m + 1) * P, :], [P, 1],
                            **_nt(f"convb{m}")) for m in range(NMT)]
            w_xp = [wtile(w_xp_d[k * P:(k + 1) * P, :], [P, DTR + 2 * DS],
                          BF, **_nt(f"w_xp{k}")) for k in range(NMT)]
            w_dt = wtile(w_dt_d[:], [DTR, DIL], BF, **_nt("w_dt"))
            dt_b = [wtile(dt_b_d[j * P:(j + 1) * P, :], [P, 1],
                          **_nt(f"dtb{j}")) for j in range(NDT)]
            A_t = [wtile(A_d[j * P:(j + 1) * P, :], [P, DS],
                         **_nt(f"A{j}")) for j in range(NDT)]
            diag_D = [wtile(diag_D_d[j], [P, P], BF, **_nt(f"dD{j}"))
                      for j in range(NDT)]
            w_out = [wtile(w_out_d[j * P:(j + 1) * P, :], [P, DIM], BF,
                           **_nt(f"w_out{j}")) for j in range(NDT)]
            n2_w = [wtile(n2_w_d[k * P:(k + 1) * P, :], [P, 1],
                          **_nt(f"n2w{k}")) for k in range(NCH)]
            n2_b = [wtile(n2_b_d[k * P:(k + 1) * P, :], [P, 1],
                          **_nt(f"n2b{k}")) for k in range(NCH)]
            ident = wtile(ident_d[:], [P, P], **_nt("ident"))
            ident_bf = wp.tile([P, P], BF, **_nt("ident_bf"))
            nc.vector.tensor_copy(ident_bf[:], ident[:])
            # einfft weights (needed latest)
            CdF = [wtile(CdF_d[t * P:(t + 1) * P, :], [P, 256], BF,
                         **_nt(f"CdF{t}")) for t in range(8)]
            SdF = [wtile(SdF_d[t * P:(t + 1) * P, :], [P, 256], BF,
                         **_nt(f"SdF{t}")) for t in range(8)]
            CdI = [wtile(CdI_d[c * P:(c + 1) * P, :], [P, L], BF,
                         **_nt(f"CdI{c}")) for c in range(2)]
            SdI = [wtile(SdI_d[c * P:(c + 1) * P, :], [P, L], BF,
                         **_nt(f"SdI{c}")) for c in range(2)]
            w1r = [wtile(w1r_d[b], [BS, BS], BF, **_nt(f"w1r{b}"))
                   for b in range(NB)]
            w1i = [wtile(w1i_d[b], [BS, BS], BF, **_nt(f"w1i{b}"))
                   for b in range(NB)]
            w1in = [wtile(w1in_d[b], [BS, BS], BF, **_nt(f"w1in{b}"))
                    for b in range(NB)]
            w2r = [wtile(w2r_d[b], [BS, BS], BF, **_nt(f"w2r{b}"))
                   for b in range(NB)]
            w2i = [wtile(w2i_d[b], [BS, BS], BF, **_nt(f"w2i{b}"))
                   for b in range(NB)]
            w2in = [wtile(w2in_d[b], [BS, BS], BF, **_nt(f"w2in{b}"))
                    for b in range(NB)]
            cb1r = [wtile(cb1r_d[b], [BS, 1], **_nt(f"cb1r{b}"))
                    for b in range(NB)]
            cb1i = [wtile(cb1i_d[b], [BS, 1], **_nt(f"cb1i{b}"))
                    for b in range(NB)]
            ssb = [[wtile(ssb_d[b, jj], [BS, 1], **_nt(f"ssb{b}_{jj}"))
                    for jj in range(4)] for b in range(NB)]

            ones_k1 = wp.tile([1, P], BF, **_nt("ones_k1"))
            nc.vector.memset(ones_k1[:], 1.0)
            ones_m1 = wp.tile([P, 1], BF, **_nt("ones_m1"))
            nc.vector.memset(ones_m1[:], 1.0)

            # ----------------------------------------------------------
            def layer_norm(w_aps, b_aps, pool, out_tag):
                """bf16 LN over partition dim (d) of x_res; returns bf16."""
                with tc.tile_pool(name="psln", bufs=1, space="PSUM") as psl:
                    pm = psl.tile([1, L], F32, **_nt("ln_pm"))
                    for k in range(NCH):
                        for h in range(2):
                            hs = slice(h * 512, (h + 1) * 512)
                            nc.tensor.matmul(pm[:, hs], ones_m1[:],
                                             x_res[k][:, hs],
                                             start=(k == 0),
                                             stop=(k == NCH - 1))
                    psq = psl.tile([1, L], F32, **_nt("ln_psq"))
                    for k in range(NCH):
                        x2 = lnp.tile([P, L], BF, **_nt("ln_x2"), bufs=2)
                        nc.scalar.activation(x2[:], x_res[k][:], AF.Square)
                        for h in range(2):
                            hs = slice(h * 512, (h + 1) * 512)
                            nc.tensor.matmul(psq[:, hs], ones_m1[:],
                                             x2[:, hs], start=(k == 0),
                                             stop=(k == NCH - 1))
                    nm = lnp.tile([1, L], BF, **_nt("ln_nm"))
                    nc.scalar.mul(nm[:], pm[:], -1.0 / DIM)
                    msq = lnp.tile([1, L], F32, **_nt("ln_msq"))
                    nc.scalar.activation(msq[:], nm[:], AF.Square)
                    ch = lnp.tile([1, L], F32, **_nt("ln_ch"))
                    nc.vector.scalar_tensor_tensor(
                        ch[:], psq[:], 1.0 / DIM, msq[:],
                        ALU.mult, ALU.subtract)
                    inv = lnp.tile([1, L], BF, **_nt("ln_inv"))
                    nc.scalar.activation(inv[:], ch[:],
                                         AF.Abs_reciprocal_sqrt, bias=1e-5)
                    nm_ps = psl.tile([P, L], F32, **_nt("ln_nmps"))
                    iv_ps = psl.tile([P, L], F32, **_nt("ln_ivps"))
                    for h in range(2):
                        hs = slice(h * 512, (h + 1) * 512)
                        nc.tensor.matmul(nm_ps[:, hs], ones_k1[:],
                                         nm[:, hs], start=True, stop=True)
                        nc.tensor.matmul(iv_ps[:, hs], ones_k1[:],
                                         inv[:, hs], start=True, stop=True)
                    nm_bc = lnp.tile([P, L], BF, **_nt("ln_nmbc"))
                    nc.scalar.copy(nm_bc[:], nm_ps[:])
                    iv_bc = lnp.tile([P, L], BF, **_nt("ln_ivbc"))
                    nc.scalar.copy(iv_bc[:], iv_ps[:])
                    outs = []
                    for k in range(NCH):
                        t1 = lnp.tile([P, L], BF, **_nt("ln_t1"), bufs=3)
                        nc.vector.tensor_tensor(t1[:], x_res[k][:],
                                                nm_bc[:], ALU.add)
                        t2 = lnp.tile([P, L], BF, **_nt("ln_t2"), bufs=3)
                        nc.vector.tensor_tensor(t2[:], t1[:], iv_bc[:],
                                                ALU.mult)
                        o = pool.tile([P, L], BF, **_nt(f"{out_tag}{k}"))
                        nc.vector.tensor_scalar(o[:], t2[:], w_aps[k][:],
                                                b_aps[k][:], ALU.mult,
                                                ALU.add)
                        outs.append(o)
                    return outs

            # ----------------------------------------------------------
            def mamba_block(blk):
                with tc.tile_pool(name="mb", bufs=1) as mb:
                    ar2_in = [dram.tile([DIM // 2, L], BF, **_nt(f"ar2i{c}"))
                              for c in range(2)]
                    ar2_out = [dram.tile([DIM // 2, L], BF,
                                         **_nt(f"ar2o{c}")) for c in range(2)]
                    projbd = dram.tile([2 * DS, L], BF, **_nt("projbd"))

                    xmc_wide = mb.tile([P, W], BF, **_nt("xmc_wide"))
                    szs_wide = mb.tile([P, W], BF, **_nt("szs_wide"))
                    dt_wide = mb.tile([P, W], F32, **_nt("dt_wide"))
                    du_wide = mb.tile([P, W], BF, **_nt("du_wide"))
                    proj_dt = mb.tile([DTR, L], BF, **_nt("proj_dt"))
                    y2 = [mb.tile([P, L], BF, **_nt(f"y2_{j}"))
                          for j in range(NDT)]

                    with tc.tile_pool(name="mpre", bufs=1) as mpre:
                        xn = layer_norm(ln_w, ln_b, mpre, "xn")
                        # --- in_proj (all 8 xm tiles + own 2 z) pipelined
                        # with the PE depthwise conv + x_proj accumulation.
                        LP = L + DC - 1
                        xm_sb = [mpre.tile([P, LP], BF, **_nt(f"xmsb{m}"))
                                 for m in range(NMT)]
                        for m in range(NMT):
                            nc.gpsimd.memset(xm_sb[m][:, 0:DC - 1], 0.0)
                        with tc.tile_pool(name="psA", bufs=2,
                                          space="PSUM") as psA:
                            for mt in range(NMT + NDT):
                                pxz = psA.tile([P, L], F32, **_nt("pxz"))
                                for k in range(NCH):
                                    lhs = w_in[k][:, mt * P:(mt + 1) * P]
                                    for h in range(2):
                                        hs = slice(h * 512, (h + 1) * 512)
                                        nc.tensor.matmul(
                                            pxz[:, hs], lhs, xn[k][:, hs],
                                            start=(k == 0),
                                            stop=(k == NCH - 1))
                                if mt < NMT:
                                    nc.vector.tensor_copy(
                                        xm_sb[mt][:, DC - 1:LP], pxz[:])
                                else:
                                    j = mt - NMT
                                    nc.scalar.activation(
                                        szs_wide[:, j * L:(j + 1) * L],
                                        pxz[:], AF.Silu)
                        with tc.tile_pool(name="psC", bufs=2,
                                          space="PSUM") as psC, \
                             tc.tile_pool(name="psP", bufs=1,
                                          space="PSUM") as psP:
                            pp1 = psP.tile([P, L], F32, **_nt("pp1"))
                            pp2 = psP.tile([32, L], F32, **_nt("pp2"))
                            xmc = [None] * NMT

                            def emit_conv(mt):
                                psc = psC.tile([P, L], F32, **_nt("psc"))
                                for q in range(DC):
                                    dg = conv_diag[mt * DC + q]
                                    for h in range(2):
                                        hs = slice(h * 512, (h + 1) * 512)
                                        nc.tensor.matmul(
                                            psc[:, hs], dg[:],
                                            xm_sb[mt][:, q + h * 512:
                                                       q + (h + 1) * 512],
                                            start=(q == 0),
                                            stop=(q == DC - 1))
                                # own-first channel order: own d-tiles are
                                # mt 0..NDT-1.
                                if mt < NDT:
                                    nc.scalar.activation(
                                        xmc_wide[:, mt * L:(mt + 1) * L],
                                        psc[:], AF.Silu,
                                        bias=conv_b[mt][:])
                                    xmc[mt] = (xmc_wide,
                                               slice(mt * L, (mt + 1) * L))
                                else:
                                    t = mpre.tile([P, L], BF,
                                                  **_nt("xmct"), bufs=4)
                                    nc.scalar.activation(t[:], psc[:],
                                                         AF.Silu,
                                                         bias=conv_b[mt][:])
                                    xmc[mt] = (t, slice(0, L))
                                src, sl = xmc[mt]
                                for h in range(2):
                                    hs = slice(h * 512, (h + 1) * 512)
                                    rhs = src[:, sl.start + h * 512:
                                              sl.start + (h + 1) * 512]
                                    nc.tensor.matmul(
                                        pp1[:, hs], w_xp[mt][:, 0:P], rhs,
                                        start=(mt == 0),
                                        stop=(mt == NMT - 1))
                                    nc.tensor.matmul(
                                        pp2[:, hs], w_xp[mt][:, P:160], rhs,
                                        start=(mt == 0),
                                        stop=(mt == NMT - 1))

                            for mt in range(NMT):
                                emit_conv(mt)
                            # extract dt-proj input + B/C rows
                            nc.vector.tensor_copy(proj_dt[:],
                                                  pp1[0:DTR, :])
                            pjA = mpre.tile([P, L], BF, **_nt("pjA"))
                            nc.vector.tensor_copy(pjA[:], pp1[:])
                            pjB = mpre.tile([32, L], BF, **_nt("pjB"))
                            nc.vector.tensor_copy(pjB[:], pp2[:])
                            nc.sync.dma_start(projbd[0:96, :],
                                              pjA[DTR:P, :])
                            nc.sync.dma_start(projbd[96:128, :], pjB[:])
                        # --- dt_proj + softplus into dt_wide halves ---
                        with tc.tile_pool(name="psD", bufs=2,
                                          space="PSUM") as psD:
                            for j in range(NDT):
                                pdt = psD.tile([P, L], F32, **_nt("pdt"))
                                for h in range(2):
                                    hs = slice(h * 512, (h + 1) * 512)
                                    nc.tensor.matmul(
                                        pdt[:, hs],
                                        w_dt[:, j * P:(j + 1) * P],
                                        proj_dt[:, hs], start=True,
                                        stop=True)
                                dtj = dt_wide[:, j * L:(j + 1) * L]
                                nc.scalar.activation(dtj, pdt[:], AF.Exp,
                                                     bias=dt_b[j][:])
                                nc.scalar.activation(dtj, dtj, AF.Ln,
                                                     bias=1.0)
                        nc.vector.tensor_tensor(du_wide[:], dt_wide[:],
                                                xmc_wide[:], ALU.mult)

                    # ------------- scan loop --------------------------
                    with tc.tile_pool(name="msc", bufs=1) as msc, \
                         tc.tile_pool(name="psY", bufs=1,
                                      space="PSUM") as psY:
                        psum_y = [psY.tile([P, L], F32, **_nt(f"py{j}"))
                                  for j in range(NDT)]
                        # init: psum_y[j] = diag(D_j) @ xmc_j
                        for j in range(NDT):
                            for h in range(2):
                                hs = slice(h * 512, (h + 1) * 512)
                                nc.tensor.matmul(
                                    psum_y[j][:, hs], diag_D[j][:],
                                    xmc_wide[:, j * L + h * 512:
                                             j * L + (h + 1) * 512],
                                    start=True, stop=False)
                        def rep3(t):
                            return t[:].unsqueeze(1).to_broadcast(
                                (P, NDT, L))

                        def wide3(t):
                            return t[:].rearrange("p (a b) -> p a b",
                                                  a=NDT)

                        for s in range(DS):
                            # one broadcast per s, shared by both d-tiles
                            bB = msc.tile([P, L], BF, **_nt("bB"), bufs=6)
                            nc.sync.dma_start(
                                bB[:],
                                projbd[s:s + 1, :].to_broadcast((P, L)))
                            bC = msc.tile([P, L], BF, **_nt("bC"), bufs=6)
                            nc.sync.dma_start(
                                bC[:],
                                projbd[DS + s:DS + s + 1,
                                       :].to_broadcast((P, L)))
                            last = (s == DS - 1)
                            # wide dB for both d-tiles in one op (stride-0
                            # repeat of the broadcast row tile)
                            dB = msc.tile([P, W], BF, **_nt("dB"), bufs=3)
                            nc.vector.tensor_tensor(
                                wide3(dB), wide3(du_wide), rep3(bB),
                                ALU.mult)
                            h_t = msc.tile([P, W], BF, **_nt("h"), bufs=3)
                            for j in range(NDT):
                                js = slice(j * L, (j + 1) * L)
                                dA = msc.tile([P, L], F32, **_nt("dA"),
                                              bufs=4)
                                nc.scalar.activation(
                                    dA[:], dt_wide[:, js], AF.Exp,
                                    scale=A_t[j][:, s:s + 1])
                                nc.vector.tensor_tensor_scan(
                                    h_t[:, js], dA[:], dB[:, js], 0.0,
                                    ALU.mult, ALU.add)
                            p = msc.tile([P, W], BF, **_nt("p"), bufs=3)
                            nc.vector.tensor_tensor(
                                wide3(p), wide3(h_t), rep3(bC), ALU.mult)
                            for j in range(NDT):
                                for h in range(2):
                                    hs = slice(h * 512, (h + 1) * 512)
                                    nc.tensor.matmul(
                                        psum_y[j][:, hs], ident_bf[:],
                                        p[:, j * L + h * 512:
                                          j * L + (h + 1) * 512],
                                        start=False, stop=last)
                        # --- gate + out_proj ---
                        for j in range(NDT):
                            nc.vector.tensor_tensor(
                                y2[j][:], psum_y[j][:],
                                szs_wide[:, j * L:(j + 1) * L], ALU.mult)
                    with tc.tile_pool(name="mpost", bufs=1) as mpost, \
                         tc.tile_pool(name="psO", bufs=2,
                                      space="PSUM") as psO:
                        for mt in range(NCH):
                            po = psO.tile([P, L], F32, **_nt("pout"))
                            for h in range(2):
                                hs = slice(h * 512, (h + 1) * 512)
                                for j in range(NDT):
                                    nc.tensor.matmul(
                                        po[:, hs],
                                        w_out[j][:, mt * P:(mt + 1) * P],
                                        y2[j][:, hs], start=(j == 0),
                                        stop=(j == NDT - 1))
                            osb = mpost.tile([P, L], BF, **_nt("ar2sb"),
                                             bufs=2)
                            nc.scalar.copy(osb[:], po[:])
                            c, rr = divmod(mt, 2)
                            nc.sync.dma_start(
                                ar2_in[c][rr * P:(rr + 1) * P, :], osb[:])
                            if rr == 1:
                                nc.gpsimd.collective_compute(
                                    "AllReduce", ALU.add,
                                    replica_groups=RG,
                                    ins=[ar2_in[c].opt()],
                                    outs=[ar2_out[c].opt()])
                        for k in range(NCH):
                            c, rr = divmod(k, 2)
                            mo = mpost.tile([P, L], BF, **_nt("mo"),
                                            bufs=2)
                            nc.sync.dma_start(
                                mo[:], ar2_out[c][rr * P:(rr + 1) * P, :])
                            nc.vector.tensor_tensor(x_res[k][:],
                                                    x_res[k][:], mo[:],
                                                    ALU.add)

            # ----------------------------------------------------------
            def bfly(pool, pl, tagp, Wb=L):
                R, I = pl[:4], pl[4:]
                t_ = {}
                for nm, (a, b, op) in {
                    "SR": (R[0], R[2], ALU.add),
                    "DR": (R[0], R[2], ALU.subtract),
                    "SR2": (R[1], R[3], ALU.add),
                    "DR2": (R[1], R[3], ALU.subtract),
                    "SI": (I[0], I[2], ALU.add),
                    "DI": (I[0], I[2], ALU.subtract),
                    "SI2": (I[1], I[3], ALU.add),
                    "DI2": (I[1], I[3], ALU.subtract),
                }.items():
                    tt = pool.tile([P, Wb], BF, **_nt(f"{tagp}t_{nm}"))
                    nc.vector.tensor_tensor(tt[:], a[:], b[:], op)
                    t_[nm] = tt
                spec = [("SR", "SR2", ALU.add), ("DR", "DI2", ALU.add),
                        ("SR", "SR2", ALU.subtract),
                        ("DR", "DI2", ALU.subtract),
                        ("SI", "SI2", ALU.add), ("DI", "DR2", ALU.subtract),
                        ("SI", "SI2", ALU.subtract), ("DI", "DR2", ALU.add)]
                out = []
                for i, (a, b, op) in enumerate(spec):
                    o = pool.tile([P, Wb], BF, **_nt(f"{tagp}o{i}"))
                    nc.vector.tensor_tensor(o[:], t_[a][:], t_[b][:], op)
                    out.append(o)
                return out[:4], out[4:]

            def einfft_block(last=False):
                KL = 256          # local k1 width
                with tc.tile_pool(name="ef", bufs=1) as ef:
                    ar3_in = [dram.tile([DIM // 2, L], BF, **_nt(f"ar3i{c}"))
                              for c in range(2)]
                    ar3_out = [dram.tile([DIM // 2, L], BF,
                                         **_nt(f"ar3o{c}")) for c in range(2)]
                    Xre = [ef.tile([P, KL], BF, **_nt(f"Xre{k}"))
                           for k in range(NCH)]
                    Xim = [ef.tile([P, KL], BF, **_nt(f"Xim{k}"))
                           for k in range(NCH)]
                    with tc.tile_pool(name="efa", bufs=1) as efa:
                        xn2 = layer_norm(n2_w, n2_b, efa, "xn2")
                        xnT = [efa.tile([P, DIM], BF, **_nt(f"xnT{t}"))
                               for t in range(8)]
                        with tc.tile_pool(name="psF", bufs=1,
                                          space="PSUM") as psF:
                            for t in range(8):
                                for k in range(NCH):
                                    pt = psF.tile([P, P], BF, **_nt("ptp"),
                                                  bufs=2)
                                    nc.tensor.transpose(
                                        pt[:], xn2[k][:, t * P:(t + 1) * P],
                                        ident_bf[:])
                                    nc.vector.tensor_copy(
                                        xnT[t][:, k * P:(k + 1) * P], pt[:])
                            for k in range(NCH):
                                pre = psF.tile([P, KL], F32, **_nt("pfr"),
                                               bufs=2)
                                pim = psF.tile([P, KL], F32, **_nt("pfi"),
                                               bufs=2)
                                for t in range(8):
                                    lhs = xnT[t][:, k * P:(k + 1) * P]
                                    nc.tensor.matmul(pre[:], lhs, CdF[t][:],
                                                     start=(t == 0),
                                                     stop=(t == 7))
                                    nc.tensor.matmul(pim[:], lhs, SdF[t][:],
                                                     start=(t == 0),
                                                     stop=(t == 7))
                                nc.vector.tensor_copy(Xre[k][:], pre[:])
                                nc.vector.tensor_scalar_mul(Xim[k][:],
                                                            pim[:], -1.0)

                    Xf_re, Xf_im = bfly(ef, Xre + Xim, "ff", KL)

                    r1 = [ef.tile([P, KL], BF, **_nt(f"r1_{b}"))
                          for b in range(NB)]
                    i1 = [ef.tile([P, KL], BF, **_nt(f"i1_{b}"))
                          for b in range(NB)]
                    with tc.tile_pool(name="psL1", bufs=2,
                                      space="PSUM") as psL1:
                        for b in range(NB):
                            pr = psL1.tile([P, KL], F32, **_nt("pl1r"))
                            nc.tensor.matmul(pr[:], w1r[b][:], Xf_re[b][:],
                                             start=True, stop=False)
                            nc.tensor.matmul(pr[:], w1in[b][:], Xf_im[b][:],
                                             start=False, stop=True)
                            nc.scalar.activation(r1[b][:], pr[:], AF.Relu,
                                                 bias=cb1r[b][:])
                            pi = psL1.tile([P, KL], F32, **_nt("pl1i"))
                            nc.tensor.matmul(pi[:], w1i[b][:], Xf_re[b][:],
                                             start=True, stop=False)
                            nc.tensor.matmul(pi[:], w1r[b][:], Xf_im[b][:],
                                             start=False, stop=True)
                            nc.scalar.activation(i1[b][:], pi[:], AF.Relu,
                                                 bias=cb1i[b][:])

                    zre = [None] * NB
                    zimN = [None] * NB
                    with tc.tile_pool(name="psL2", bufs=2,
                                      space="PSUM") as psL2:
                        for b in range(NB):
                            pzr = psL2.tile([P, KL], F32, **_nt("pl2r"))
                            nc.tensor.matmul(pzr[:], w2r[b][:], r1[b][:],
                                             start=True, stop=False)
                            nc.tensor.matmul(pzr[:], w2in[b][:], i1[b][:],
                                             start=False, stop=True)
                            a1 = ef.tile([P, KL], BF, **_nt("ss"), bufs=4)
                            nc.scalar.activation(a1[:], pzr[:], AF.Relu,
                                                 scale=0.5, bias=ssb[b][0][:])
                            a2 = ef.tile([P, KL], BF, **_nt("ss"), bufs=4)
                            nc.scalar.activation(a2[:], pzr[:], AF.Relu,
                                                 scale=-0.5,
                                                 bias=ssb[b][1][:])
                            zr = ef.tile([P, KL], BF, name=f"zre{b}",
                                         tag=f"Xre{b}")
                            nc.vector.tensor_tensor(zr[:], a1[:], a2[:],
                                                    ALU.subtract)
                            zre[b] = zr
                            pzi = psL2.tile([P, KL], F32, **_nt("pl2i"))
                            nc.tensor.matmul(pzi[:], w2i[b][:], r1[b][:],
                                             start=True, stop=False)
                            nc.tensor.matmul(pzi[:], w2r[b][:], i1[b][:],
                                             start=False, stop=True)
                            b1 = ef.tile([P, KL], BF, **_nt("ss"), bufs=4)
                            nc.scalar.activation(b1[:], pzi[:], AF.Relu,
                                                 scale=0.5, bias=ssb[b][2][:])
                            b2 = ef.tile([P, KL], BF, **_nt("ss"), bufs=4)
                            nc.scalar.activation(b2[:], pzi[:], AF.Relu,
                                                 scale=-0.5,
                                                 bias=ssb[b][3][:])
                            zi = ef.tile([P, KL], BF, name=f"zimN{b}",
                                         tag=f"Xim{b}")
                            nc.vector.tensor_tensor(zi[:], b2[:], b1[:],
                                                    ALU.subtract)
                            zimN[b] = zi

                    zz_re, zz_iN = bfly(ef, zre + zimN, "ff", KL)

                    with tc.tile_pool(name="psI", bufs=2,
                                      space="PSUM") as psI:
                        for b in range(NB):
                            zTr = ef.tile([P, KL], BF, **_nt("zzTr"),
                                          bufs=2)
                            zTi = ef.tile([P, KL], BF, **_nt("zzTi"),
                                          bufs=2)
                            for c in range(2):
                                pt = psI.tile([P, P], BF, **_nt("ptp2"))
                                nc.tensor.transpose(
                                    pt[:], zz_re[b][:, c * P:(c + 1) * P],
                                    ident_bf[:])
                                nc.vector.tensor_copy(
                                    zTr[:, c * P:(c + 1) * P], pt[:])
                                pt2 = psI.tile([P, P], BF, **_nt("ptp3"))
                                nc.tensor.transpose(
                                    pt2[:], zz_iN[b][:, c * P:(c + 1) * P],
                                    ident_bf[:])
                                nc.vector.tensor_copy(
                                    zTi[:, c * P:(c + 1) * P], pt2[:])
                            for h in range(2):
                                hs = slice(h * 512, (h + 1) * 512)
                                pout = psI.tile([P, 512], F32,
                                                **_nt("pidft"))
                                for c in range(2):
                                    nc.tensor.matmul(
                                        pout[:], zTr[:, c * P:(c + 1) * P],
                                        CdI[c][:, hs], start=(c == 0),
                                        stop=False)
                                    nc.tensor.matmul(
                                        pout[:], zTi[:, c * P:(c + 1) * P],
                                        SdI[c][:, hs], start=False,
                                        stop=(c == 1))
                                ob = ef.tile([P, 512], BF, **_nt("eob"),
                                             bufs=3)
                                nc.vector.tensor_copy(ob[:], pout[:])
                                if last:
                                    nc.sync.dma_start(
                                        xP_d[b * P:(b + 1) * P, hs], ob[:])
                                else:
                                    c, rr = divmod(b, 2)
                                    nc.sync.dma_start(
                                        ar3_in[c][rr * P:(rr + 1) * P, hs],
                                        ob[:])
                            if not last and b % 2 == 1:
                                c = b // 2
                                nc.gpsimd.collective_compute(
                                    "AllReduce", ALU.add,
                                    replica_groups=RG,
                                    ins=[ar3_in[c].opt()],
                                    outs=[ar3_out[c].opt()])

                    if not last:
                        for k in range(NCH):
                            c, rr = divmod(k, 2)
                            eo = ef.tile([P, L], BF, **_nt("eo"), bufs=2)
                            nc.sync.dma_start(
                                eo[:], ar3_out[c][rr * P:(rr + 1) * P, :])
                            nc.vector.tensor_tensor(x_res[k][:],
                                                    x_res[k][:], eo[:],
                                                    ALU.add)

            for blk in range(BLOCKS):
                mamba_block(blk)
                if blk == BLOCKS - 1:
                    for k in range(NCH):
                        nc.sync.dma_start(xO_d[k * P:(k + 1) * P, :],
                                          x_res[k][:])
                einfft_block(last=(blk == BLOCKS - 1))

    nc.compile()
    return nc


# --------------------------------------------------------------------------

def _make_inmaps(inputs):
    f32 = np.float32
    x = np.asarray(inputs["x"], f32)
    in_proj_w = np.asarray(inputs["in_proj_w"], f32)
    conv_w = np.asarray(inputs["conv_w"], f32)
    conv_b = np.asarray(inputs["conv_b"], f32)
    x_proj_w = np.asarray(inputs["x_proj_w"], f32)
    dt_proj_w = np.asarray(inputs["dt_proj_w"], f32)
    dt_proj_b = np.asarray(inputs["dt_proj_b"], f32)
    A_log = np.asarray(inputs["A_log"], f32)
    Dvec = np.asarray(inputs["D"], f32)
    out_proj_w = np.asarray(inputs["out_proj_w"], f32)
    ln_w = np.asarray(inputs["ln_w"], f32)
    ln_b = np.asarray(inputs["ln_b"], f32)
    n2_w = np.asarray(inputs["norm2_w"], f32)
    n2_b = np.asarray(inputs["norm2_b"], f32)
    cw1 = np.asarray(inputs["cw1"], f32)
    cw2 = np.asarray(inputs["cw2"], f32)
    cb1 = np.asarray(inputs["cb1"], f32)
    cb2 = np.asarray(inputs["cb2"], f32)

    n = np.arange(L, dtype=np.float64)
    ang = 2.0 * np.pi * np.outer(n, n) / L
    Cdft = (np.cos(ang) / np.sqrt(L)).astype(BF16)
    Sdft = (np.sin(ang) / np.sqrt(L)).astype(BF16)
    CdF = [np.ascontiguousarray(Cdft[:, r * 256:(r + 1) * 256])
           for r in range(GROUP)]
    SdF = [np.ascontiguousarray(Sdft[:, r * 256:(r + 1) * 256])
           for r in range(GROUP)]
    CdI = [np.ascontiguousarray(Cdft[r * 256:(r + 1) * 256, :])
           for r in range(GROUP)]
    SdI = [np.ascontiguousarray(Sdft[r * 256:(r + 1) * 256, :])
           for r in range(GROUP)]

    ssb = np.stack([
        (cb2[0] - LAM) / 2, (-cb2[0] - LAM) / 2,
        (cb2[1] - LAM) / 2, (-cb2[1] - LAM) / 2,
    ], axis=1)[:, :, :, None]

    in_maps = []
    for core in range(N_CORES):
        g, r = divmod(core, GROUP)
        lo, hi = r * DIL, (r + 1) * DIL
        # per-core channel order: own d_inner slice first, then the rest
        perm = np.r_[lo:hi, 0:lo, hi:DI]
        conv_wp = conv_w[perm, 0, :]
        conv_diag = np.zeros((NMT * DC, P, P), f32)
        for mt in range(NMT):
            for q in range(DC):
                np.fill_diagonal(conv_diag[mt * DC + q],
                                 conv_wp[mt * P:(mt + 1) * P, q])
        diag_D = np.zeros((NDT, P, P), f32)
        for j in range(NDT):
            np.fill_diagonal(diag_D[j], Dvec[lo + j * P: lo + (j + 1) * P])
        m = {
            "xT": np.ascontiguousarray(x[g].T).astype(BF16),
            "w_in": np.ascontiguousarray(
                np.concatenate([in_proj_w[perm],
                                in_proj_w[DI + lo:DI + hi]], 0).T
            ).astype(BF16),
            "conv_diag": conv_diag.astype(BF16),
            "conv_b": np.ascontiguousarray(conv_b[perm][:, None]),
            "w_xp": np.ascontiguousarray(x_proj_w[:, perm].T).astype(BF16),
            "w_dt": np.ascontiguousarray(dt_proj_w[lo:hi].T).astype(BF16),
            "dt_b": np.ascontiguousarray(dt_proj_b[lo:hi][:, None]),
            "A": np.ascontiguousarray(-np.exp(A_log[lo:hi])),
            "diag_D": diag_D.astype(BF16),
            "w_out": np.ascontiguousarray(
                out_proj_w[:, lo:hi].T).astype(BF16),
            "ln_w": np.ascontiguousarray(ln_w[:, None]),
            "ln_b": np.ascontiguousarray(ln_b[:, None]),
            "n2_w": np.ascontiguousarray(n2_w[:, None]),
            "n2_b": np.ascontiguousarray(n2_b[:, None]),
            "CdF": CdF[r], "SdF": SdF[r],
            "CdI": CdI[r], "SdI": SdI[r],
            "w1r": (0.5 * cw1[0]).astype(BF16),
            "w1i": (0.5 * cw1[1]).astype(BF16),
            "w1in": (-0.5 * cw1[1]).astype(BF16),
            "w2r": cw2[0].astype(BF16),
            "w2i": cw2[1].astype(BF16),
            "w2in": (-cw2[1]).astype(BF16),
            "cb1r": np.ascontiguousarray(cb1[0][:, :, None]),
            "cb1i": np.ascontiguousarray(cb1[1][:, :, None]),
            "ssb": np.ascontiguousarray(ssb, f32),
            "ident": np.eye(P, dtype=f32),
        }
        in_maps.append(m)
    return in_maps


def kernel(**inputs):
    global _COMPILED
    from concourse.bass_utils import run_bass_kernel_spmd
    if _COMPILED is None:
        _COMPILED = _build_program()
    in_maps = _make_inmaps(inputs)
    res = run_bass_kernel_spmd(_COMPILED, in_maps,
                               core_ids=list(range(N_CORES)))
    outs = []
    for g in range(2):
        x = res.results[g * GROUP]["xO"].astype(np.float32)
        for r in range(GROUP):
            x = x + res.results[g * GROUP + r]["xP"].astype(np.float32)
        outs.append(x.T)
    return np.ascontiguousarray(np.stack(outs).astype(np.float32))


# revision 29
# speedup vs baseline: 1.0180x; 1.0018x over previous
"""Trainium2 Bass kernel for nn_Block_mamba (SiMBA-style block: Mamba + EinFFT).

8 NeuronCores = 2 batch groups x 4-way shard of d_inner (256 ch/core).
v2 design notes (vector engine is the bottleneck; scans are a fixed
~2.1ns/elem DVE cost, dtype independent):
 - in_proj/conv replicated per core (no AllGather); depthwise conv runs on
   the PE as diagonal matmuls with shifted column windows.
 - selective scan processes both local d-tiles packed in one wide
   [128, 2048] free dim; the j-boundary column of dA is zeroed so one
   hardware scan covers both tiles.
 - dA stays fp32 (scan speed is dtype independent); dB/h/p bf16.
 - p = h*C runs on GpSimd; the s-accumulation is PE matmul-accumulate with
   an identity stationary into PSUM (fp32, exact), D*xm folded in via a
   diagonal-D matmul.
 - residual stream bf16; LN uses PE for stats/broadcast, tensor_scalar 4x.

kernel(**inputs): full unsharded inputs -> full (2, 1024, 512) output.
"""

import numpy as np
import ml_dtypes

DIM = 512
NB = 4
BS = 128
DS = 64
DC = 4
DI = 1024
DTR = 32
BLOCKS = 2
LAM = 0.01
L = 1024

N_CORES = 8
GROUP = 4
DIL = DI // GROUP       # 256
P = 128
NDT = DIL // P          # 2
NCH = DIM // P          # 4
NMT = DI // P           # 8 xm tiles (replicated)
W = 2 * L               # wide free dim (both d-tiles packed)

BF16 = ml_dtypes.bfloat16

_COMPILED = None


def _nt(s):
    return {"name": s, "tag": s}


def _build_program():
    import contextlib
    import concourse.bacc as bacc
    import concourse.mybir as mybir
    import concourse.tile as tile

    F32 = mybir.dt.float32
    BF = mybir.dt.bfloat16
    AF = mybir.ActivationFunctionType
    ALU = mybir.AluOpType

    nc = bacc.Bacc("TRN2", target_bir_lowering=False, debug=False,
                   num_devices=N_CORES)

    _eps = nc.alloc_sbuf_tensor("const-float32-eps", [128, 1], F32)
    nc.gpsimd.memset(_eps.ap(), 1e-5)
    nc.const_aps.aps[(F32, 1e-5)] = _eps.ap()
    nc.all_engine_barrier()

    def din(name, shape, dt=F32):
        return nc.dram_tensor(name, shape, dt, kind="ExternalInput")

    xT_d = din("xT", [DIM, L], BF)
    w_in_d = din("w_in", [DIM, DI + DIL], BF)
    conv_diag_d = din("conv_diag", [NMT * DC, P, P], BF)
    conv_b_d = din("conv_b", [DI, 1])
    w_xp_d = din("w_xp", [DI, DTR + 2 * DS], BF)
    w_dt_d = din("w_dt", [DTR, DIL], BF)
    dt_b_d = din("dt_b", [DIL, 1])
    A_d = din("A", [DIL, DS])
    diag_D_d = din("diag_D", [NDT, P, P], BF)
    w_out_d = din("w_out", [DIL, DIM], BF)
    ln_w_d = din("ln_w", [DIM, 1])
    ln_b_d = din("ln_b", [DIM, 1])
    n2_w_d = din("n2_w", [DIM, 1])
    n2_b_d = din("n2_b", [DIM, 1])
    CdF_d = din("CdF", [L, 256], BF)    # C[:, k1_loc] forward
    SdF_d = din("SdF", [L, 256], BF)
    CdI_d = din("CdI", [256, L], BF)    # C[k1_loc, :] inverse
    SdI_d = din("SdI", [256, L], BF)
    w1r_d = din("w1r", [NB, BS, BS], BF)
    w1i_d = din("w1i", [NB, BS, BS], BF)
    w1in_d = din("w1in", [NB, BS, BS], BF)
    w2r_d = din("w2r", [NB, BS, BS], BF)
    w2i_d = din("w2i", [NB, BS, BS], BF)
    w2in_d = din("w2in", [NB, BS, BS], BF)
    cb1r_d = din("cb1r", [NB, BS, 1])
    cb1i_d = din("cb1i", [NB, BS, 1])
    ssb_d = din("ssb", [NB, 4, BS, 1])
    ident_d = din("ident", [P, P])
    xO_d = nc.dram_tensor("xO", [DIM, L], BF, kind="ExternalOutput")
    xP_d = nc.dram_tensor("xP", [DIM, L], BF, kind="ExternalOutput")

    RG = [[0, 1, 2, 3], [4, 5, 6, 7]]

    with tile.TileContext(nc) as tc:
        stack = contextlib.ExitStack()
        with stack:
            wp = stack.enter_context(tc.tile_pool(name="wp", bufs=1))
            ap = stack.enter_context(tc.tile_pool(name="ap", bufs=1))
            lnp = stack.enter_context(tc.tile_pool(name="lnp", bufs=1))
            dram = stack.enter_context(
                tc.tile_pool(name="dram", bufs=1, space="DRAM"))

            # residual stream (bf16)
            x_res = [ap.tile([P, L], BF, **_nt(f"xres{k}"))
                     for k in range(NCH)]
            for k in range(NCH):
                nc.sync.dma_start(x_res[k][:], xT_d[k * P:(k + 1) * P, :])

            def wtile(src, shape, dt=F32, name=None, tag=None):
                t = wp.tile(shape, dt, name=name, tag=tag)
                nc.sync.dma_start(t[:], src)
                return t

            # critical-path weights first
            ln_w = [wtile(ln_w_d[k * P:(k + 1) * P, :], [P, 1],
                          **_nt(f"lnw{k}")) for k in range(NCH)]
            ln_b = [wtile(ln_b_d[k * P:(k + 1) * P, :], [P, 1],
                          **_nt(f"lnb{k}")) for k in range(NCH)]
            w_in = [wtile(w_in_d[k * P:(k + 1) * P, :], [P, DI + DIL], BF,
                          **_nt(f"w_in{k}")) for k in range(NCH)]
            conv_diag = [wtile(conv_diag_d[i], [P, P], BF,
                               **_nt(f"cvd{i}")) for i in range(NMT * DC)]
            conv_b = [wtile(conv_b_d[m * P:(m + 1) * P, :], [P, 1],
                            **_nt(f"convb{m}")) for m in range(NMT)]
            w_xp = [wtile(w_xp_d[k * P:(k + 1) * P, :], [P, DTR + 2 * DS],
                          BF, **_nt(f"w_xp{k}")) for k in range(NMT)]
            w_dt = wtile(w_dt_d[:], [DTR, DIL], BF, **_nt("w_dt"))
            dt_b = [wtile(dt_b_d[j * P:(j + 1) * P, :], [P, 1],
                          **_nt(f"dtb{j}")) for j in range(NDT)]
            A_t = [wtile(A_d[j * P:(j + 1) * P, :], [P, DS],
                         **_nt(f"A{j}")) for j in range(NDT)]
            diag_D = [wtile(diag_D_d[j], [P, P], BF, **_nt(f"dD{j}"))
                      for j in range(NDT)]
            w_out = [wtile(w_out_d[j * P:(j + 1) * P, :], [P, DIM], BF,
                           **_nt(f"w_out{j}")) for j in range(NDT)]
            n2_w = [wtile(n2_w_d[k * P:(k + 1) * P, :], [P, 1],
                          **_nt(f"n2w{k}")) for k in range(NCH)]
            n2_b = [wtile(n2_b_d[k * P:(k + 1) * P, :], [P, 1],
                          **_nt(f"n2b{k}")) for k in range(NCH)]
            ident = wtile(ident_d[:], [P, P], **_nt("ident"))
            ident_bf = wp.tile([P, P], BF, **_nt("ident_bf"))
            nc.vector.tensor_copy(ident_bf[:], ident[:])
            # einfft weights (needed latest)
            CdF = [wtile(CdF_d[t * P:(t + 1) * P, :], [P, 256], BF,
                         **_nt(f"CdF{t}")) for t in range(8)]
            SdF = [wtile(SdF_d[t * P:(t + 1) * P, :], [P, 256], BF,
                         **_nt(f"SdF{t}")) for t in range(8)]
            CdI = [wtile(CdI_d[c * P:(c + 1) * P, :], [P, L], BF,
                         **_nt(f"CdI{c}")) for c in range(2)]
            SdI = [wtile(SdI_d[c * P:(c + 1) * P, :], [P, L], BF,
                         **_nt(f"SdI{c}")) for c in range(2)]
            w1r = [wtile(w1r_d[b], [BS, BS], BF, **_nt(f"w1r{b}"))
                   for b in range(NB)]
            w1i = [wtile(w1i_d[b], [BS, BS], BF, **_nt(f"w1i{b}"))
                   for b in range(NB)]
            w1in = [wtile(w1in_d[b], [BS, BS], BF, **_nt(f"w1in{b}"))
                    for b in range(NB)]
            w2r = [wtile(w2r_d[b], [BS, BS], BF, **_nt(f"w2r{b}"))
                   for b in range(NB)]
            w2i = [wtile(w2i_d[b], [BS, BS], BF, **_nt(f"w2i{b}"))
                   for b in range(NB)]
            w2in = [wtile(w2in_d[b], [BS, BS], BF, **_nt(f"w2in{b}"))
                    for b in range(NB)]
            cb1r = [wtile(cb1r_d[b], [BS, 1], **_nt(f"cb1r{b}"))
                    for b in range(NB)]
            cb1i = [wtile(cb1i_d[b], [BS, 1], **_nt(f"cb1i{b}"))
                    for b in range(NB)]
            ssb = [[wtile(ssb_d[b, jj], [BS, 1], **_nt(f"ssb{b}_{jj}"))
                    for jj in range(4)] for b in range(NB)]

            ones_k1 = wp.tile([1, P], BF, **_nt("ones_k1"))
            nc.vector.memset(ones_k1[:], 1.0)
            ones_m1 = wp.tile([P, 1], BF, **_nt("ones_m1"))
            nc.vector.memset(ones_m1[:], 1.0)

            # ----------------------------------------------------------
            def layer_norm(w_aps, b_aps, pool, out_tag):
                """bf16 LN over partition dim (d) of x_res; returns bf16."""
                with tc.tile_pool(name="psln", bufs=1, space="PSUM") as psl:
                    pm = psl.tile([1, L], F32, **_nt("ln_pm"))
                    for k in range(NCH):
                        for h in range(2):
                            hs = slice(h * 512, (h + 1) * 512)
                            nc.tensor.matmul(pm[:, hs], ones_m1[:],
                                             x_res[k][:, hs],
                                             start=(k == 0),
                                             stop=(k == NCH - 1))
                    psq = psl.tile([1, L], F32, **_nt("ln_psq"))
                    for k in range(NCH):
                        x2 = lnp.tile([P, L], BF, **_nt("ln_x2"), bufs=2)
                        nc.scalar.activation(x2[:], x_res[k][:], AF.Square)
                        for h in range(2):
                            hs = slice(h * 512, (h + 1) * 512)
                            nc.tensor.matmul(psq[:, hs], ones_m1[:],
                                             x2[:, hs], start=(k == 0),
                                             stop=(k == NCH - 1))
                    nm = lnp.tile([1, L], BF, **_nt("ln_nm"))
                    nc.scalar.mul(nm[:], pm[:], -1.0 / DIM)
                    msq = lnp.tile([1, L], F32, **_nt("ln_msq"))
                    nc.scalar.activation(msq[:], pm[:], AF.Square,
                                         scale=1.0 / DIM)
                    ch = lnp.tile([1, L], F32, **_nt("ln_ch"))
                    nc.vector.scalar_tensor_tensor(
                        ch[:], psq[:], 1.0 / DIM, msq[:],
                        ALU.mult, ALU.subtract)
                    inv = lnp.tile([1, L], BF, **_nt("ln_inv"))
                    nc.scalar.activation(inv[:], ch[:],
                                         AF.Abs_reciprocal_sqrt, bias=1e-5)
                    nm_ps = psl.tile([P, L], F32, **_nt("ln_nmps"))
                    iv_ps = psl.tile([P, L], F32, **_nt("ln_ivps"))
                    for h in range(2):
                        hs = slice(h * 512, (h + 1) * 512)
                        nc.tensor.matmul(nm_ps[:, hs], ones_k1[:],
                                         nm[:, hs], start=True, stop=True)
                        nc.tensor.matmul(iv_ps[:, hs], ones_k1[:],
                                         inv[:, hs], start=True, stop=True)
                    nm_bc = lnp.tile([P, L], BF, **_nt("ln_nmbc"))
                    nc.scalar.copy(nm_bc[:], nm_ps[:])
                    iv_bc = lnp.tile([P, L], BF, **_nt("ln_ivbc"))
                    nc.scalar.copy(iv_bc[:], iv_ps[:])
                    outs = []
                    for k in range(NCH):
                        t1 = lnp.tile([P, L], BF, **_nt("ln_t1"), bufs=3)
                        nc.vector.tensor_tensor(t1[:], x_res[k][:],
                                                nm_bc[:], ALU.add)
                        t2 = lnp.tile([P, L], BF, **_nt("ln_t2"), bufs=3)
                        nc.vector.tensor_tensor(t2[:], t1[:], iv_bc[:],
                                                ALU.mult)
                        o = pool.tile([P, L], BF, **_nt(f"{out_tag}{k}"))
                        nc.vector.tensor_scalar(o[:], t2[:], w_aps[k][:],
                                                b_aps[k][:], ALU.mult,
                                                ALU.add)
                        outs.append(o)
                    return outs

            # ----------------------------------------------------------
            def mamba_block(blk):
                with tc.tile_pool(name="mb", bufs=1) as mb:
                    ar2_in = [dram.tile([DIM // 2, L], BF, **_nt(f"ar2i{c}"))
                              for c in range(2)]
                    ar2_out = [dram.tile([DIM // 2, L], BF,
                                         **_nt(f"ar2o{c}")) for c in range(2)]
                    projbd = dram.tile([2 * DS, L], BF, **_nt("projbd"))

                    xmc_wide = mb.tile([P, W], BF, **_nt("xmc_wide"))
                    szs_wide = mb.tile([P, W], BF, **_nt("szs_wide"))
                    dt_wide = mb.tile([P, W], F32, **_nt("dt_wide"))
                    du_wide = mb.tile([P, W], BF, **_nt("du_wide"))
                    proj_dt = mb.tile([DTR, L], BF, **_nt("proj_dt"))
                    y2 = [mb.tile([P, L], BF, **_nt(f"y2_{j}"))
                          for j in range(NDT)]

                    with tc.tile_pool(name="mpre", bufs=1) as mpre:
                        xn = layer_norm(ln_w, ln_b, mpre, "xn")
                        # --- in_proj (all 8 xm tiles + own 2 z) pipelined
                        # with the PE depthwise conv + x_proj accumulation.
                        LP = L + DC - 1
                        xm_sb = [mpre.tile([P, LP], BF, **_nt(f"xmsb{m}"))
                                 for m in range(NMT)]
                        for m in range(NMT):
                            nc.gpsimd.memset(xm_sb[m][:, 0:DC - 1], 0.0)
                        with tc.tile_pool(name="psA", bufs=3,
                                          space="PSUM") as psA:
                            for mt in range(NMT + NDT):
                                pxz = psA.tile([P, L], F32, **_nt("pxz"))
                                for k in range(NCH):
                                    lhs = w_in[k][:, mt * P:(mt + 1) * P]
                                    for h in range(2):
                                        hs = slice(h * 512, (h + 1) * 512)
                                        nc.tensor.matmul(
                                            pxz[:, hs], lhs, xn[k][:, hs],
                                            start=(k == 0),
                                            stop=(k == NCH - 1))
                                if mt < NMT:
                                    nc.vector.tensor_copy(
                                        xm_sb[mt][:, DC - 1:LP], pxz[:])
                                else:
                                    j = mt - NMT
                                    nc.scalar.activation(
                                        szs_wide[:, j * L:(j + 1) * L],
                                        pxz[:], AF.Silu)
                        with tc.tile_pool(name="psC", bufs=2,
                                          space="PSUM") as psC, \
                             tc.tile_pool(name="psP", bufs=1,
                                          space="PSUM") as psP:
                            pp1 = psP.tile([P, L], F32, **_nt("pp1"))
                            pp2 = psP.tile([32, L], F32, **_nt("pp2"))
                            xmc = [None] * NMT

                            def emit_conv(mt):
                                psc = psC.tile([P, L], F32, **_nt("psc"))
                                for q in range(DC):
                                    dg = conv_diag[mt * DC + q]
                                    for h in range(2):
                                        hs = slice(h * 512, (h + 1) * 512)
                                        nc.tensor.matmul(
                                            psc[:, hs], dg[:],
                                            xm_sb[mt][:, q + h * 512:
                                                       q + (h + 1) * 512],
                                            start=(q == 0),
                                            stop=(q == DC - 1))
                                # own-first channel order: own d-tiles are
                                # mt 0..NDT-1.
                                if mt < NDT:
                                    nc.scalar.activation(
                                        xmc_wide[:, mt * L:(mt + 1) * L],
                                        psc[:], AF.Silu,
                                        bias=conv_b[mt][:])
                                    xmc[mt] = (xmc_wide,
                                               slice(mt * L, (mt + 1) * L))
                                else:
                                    t = mpre.tile([P, L], BF,
                                                  **_nt("xmct"), bufs=4)
                                    nc.scalar.activation(t[:], psc[:],
                                                         AF.Silu,
                                                         bias=conv_b[mt][:])
                                    xmc[mt] = (t, slice(0, L))

                            def emit_xproj(mt):
                                src, sl = xmc[mt]
                                for h in range(2):
                                    hs = slice(h * 512, (h + 1) * 512)
                                    rhs = src[:, sl.start + h * 512:
                                              sl.start + (h + 1) * 512]
                                    nc.tensor.matmul(
                                        pp1[:, hs], w_xp[mt][:, 0:P], rhs,
                                        start=(mt == 0),
                                        stop=(mt == NMT - 1))
                                    nc.tensor.matmul(
                                        pp2[:, hs], w_xp[mt][:, P:160], rhs,
                                        start=(mt == 0),
                                        stop=(mt == NMT - 1))

                            # emit conv(mt+1) before xproj(mt) so the PE
                            # never stalls on the silu between them
                            emit_conv(0)
                            for mt in range(1, NMT):
                                emit_conv(mt)
                                emit_xproj(mt - 1)
                            emit_xproj(NMT - 1)
                            # extract dt-proj input + B/C rows
                            nc.vector.tensor_copy(proj_dt[:],
                                                  pp1[0:DTR, :])
                            pjA = mpre.tile([P, L], BF, **_nt("pjA"))
                            nc.vector.tensor_copy(pjA[:], pp1[:])
                            pjB = mpre.tile([32, L], BF, **_nt("pjB"))
                            nc.vector.tensor_copy(pjB[:], pp2[:])
                            nc.sync.dma_start(projbd[0:96, :],
                                              pjA[DTR:P, :])
                            nc.sync.dma_start(projbd[96:128, :], pjB[:])
                        # --- dt_proj + softplus into dt_wide halves ---
                        with tc.tile_pool(name="psD", bufs=2,
                                          space="PSUM") as psD:
                            for j in range(NDT):
                                pdt = psD.tile([P, L], F32, **_nt("pdt"))
                                for h in range(2):
                                    hs = slice(h * 512, (h + 1) * 512)
                                    nc.tensor.matmul(
                                        pdt[:, hs],
                                        w_dt[:, j * P:(j + 1) * P],
                                        proj_dt[:, hs], start=True,
                                        stop=True)
                                dtj = dt_wide[:, j * L:(j + 1) * L]
                                nc.scalar.activation(dtj, pdt[:], AF.Exp,
                                                     bias=dt_b[j][:])
                                nc.scalar.activation(dtj, dtj, AF.Ln,
                                                     bias=1.0)
                        nc.vector.tensor_tensor(du_wide[:], dt_wide[:],
                                                xmc_wide[:], ALU.mult)

                    # ------------- scan loop --------------------------
                    with tc.tile_pool(name="msc", bufs=1) as msc, \
                         tc.tile_pool(name="psY", bufs=1,
                                      space="PSUM") as psY:
                        psum_y = [psY.tile([P, L], F32, **_nt(f"py{j}"))
                                  for j in range(NDT)]
                        # init: psum_y[j] = diag(D_j) @ xmc_j
                        for j in range(NDT):
                            for h in range(2):
                                hs = slice(h * 512, (h + 1) * 512)
                                nc.tensor.matmul(
                                    psum_y[j][:, hs], diag_D[j][:],
                                    xmc_wide[:, j * L + h * 512:
                                             j * L + (h + 1) * 512],
                                    start=True, stop=False)
                        def rep3(t):
                            return t[:].unsqueeze(1).to_broadcast(
                                (P, NDT, L))

                        def wide3(t):
                            return t[:].rearrange("p (a b) -> p a b",
                                                  a=NDT)

                        for s in range(DS):
                            # one broadcast per s, shared by both d-tiles
                            bB = msc.tile([P, L], BF, **_nt("bB"), bufs=6)
                            nc.sync.dma_start(
                                bB[:],
                                projbd[s:s + 1, :].to_broadcast((P, L)))
                            bC = msc.tile([P, L], BF, **_nt("bC"), bufs=6)
                            nc.sync.dma_start(
                                bC[:],
                                projbd[DS + s:DS + s + 1,
                                       :].to_broadcast((P, L)))
                            last = (s == DS - 1)
                            # wide dB for both d-tiles in one op (stride-0
                            # repeat of the broadcast row tile)
                            dB = msc.tile([P, W], BF, **_nt("dB"), bufs=3)
                            nc.vector.tensor_tensor(
                                wide3(dB), wide3(du_wide), rep3(bB),
                                ALU.mult)
                            h_t = msc.tile([P, W], BF, **_nt("h"), bufs=3)
                            for j in range(NDT):
                                js = slice(j * L, (j + 1) * L)
                                dA = msc.tile([P, L], F32, **_nt("dA"),
                                              bufs=4)
                                nc.scalar.activation(
                                    dA[:], dt_wide[:, js], AF.Exp,
                                    scale=A_t[j][:, s:s + 1])
                                nc.vector.tensor_tensor_scan(
                                    h_t[:, js], dA[:], dB[:, js], 0.0,
                                    ALU.mult, ALU.add)
                            p = msc.tile([P, W], BF, **_nt("p"), bufs=3)
                            nc.vector.tensor_tensor(
                                wide3(p), wide3(h_t), rep3(bC), ALU.mult)
                            for j in range(NDT):
                                for h in range(2):
                                    hs = slice(h * 512, (h + 1) * 512)
                                    nc.tensor.matmul(
                                        psum_y[j][:, hs], ident_bf[:],
                                        p[:, j * L + h * 512:
                                          j * L + (h + 1) * 512],
                                        start=False, stop=last)
                        # --- gate + out_proj ---
                        for j in range(NDT):
                            nc.vector.tensor_tensor(
                                y2[j][:], psum_y[j][:],
                                szs_wide[:, j * L:(j + 1) * L], ALU.mult)
                    with tc.tile_pool(name="mpost", bufs=1) as mpost, \
                         tc.tile_pool(name="psO", bufs=2,
                                      space="PSUM") as psO:
                        for mt in range(NCH):
                            po = psO.tile([P, L], F32, **_nt("pout"))
                            for h in range(2):
                                hs = slice(h * 512, (h + 1) * 512)
                                for j in range(NDT):
                                    nc.tensor.matmul(
                                        po[:, hs],
                                        w_out[j][:, mt * P:(mt + 1) * P],
                                        y2[j][:, hs], start=(j == 0),
                                        stop=(j == NDT - 1))
                            osb = mpost.tile([P, L], BF, **_nt("ar2sb"),
                                             bufs=2)
                            nc.scalar.copy(osb[:], po[:])
                            c, rr = divmod(mt, 2)
                            nc.sync.dma_start(
                                ar2_in[c][rr * P:(rr + 1) * P, :], osb[:])
                            if rr == 1:
                                nc.gpsimd.collective_compute(
                                    "AllReduce", ALU.add,
                                    replica_groups=RG,
                                    ins=[ar2_in[c].opt()],
                                    outs=[ar2_out[c].opt()])
                        for k in range(NCH):
                            c, rr = divmod(k, 2)
                            mo = mpost.tile([P, L], BF, **_nt("mo"),
                                            bufs=2)
                            nc.sync.dma_start(
                                mo[:], ar2_out[c][rr * P:(rr + 1) * P, :])
                            nc.vector.tensor_tensor(x_res[k][:],
                                                    x_res[k][:], mo[:],
                                                    ALU.add)

            # ----------------------------------------------------------
            def bfly(pool, pl, tagp, Wb=L):
                R, I = pl[:4], pl[4:]
                t_ = {}
                for nm, (a, b, op) in {
                    "SR": (R[0], R[2], ALU.add),
                    "DR": (R[0], R[2], ALU.subtract),
                    "SR2": (R[1], R[3], ALU.add),
                    "DR2": (R[1], R[3], ALU.subtract),
                    "SI": (I[0], I[2], ALU.add),
                    "DI": (I[0], I[2], ALU.subtract),
                    "SI2": (I[1], I[3], ALU.add),
                    "DI2": (I[1], I[3], ALU.subtract),
                }.items():
                    tt = pool.tile([P, Wb], BF, **_nt(f"{tagp}t_{nm}"))
                    nc.vector.tensor_tensor(tt[:], a[:], b[:], op)
                    t_[nm] = tt
                spec = [("SR", "SR2", ALU.add), ("DR", "DI2", ALU.add),
                        ("SR", "SR2", ALU.subtract),
                        ("DR", "DI2", ALU.subtract),
                        ("SI", "SI2", ALU.add), ("DI", "DR2", ALU.subtract),
                        ("SI", "SI2", ALU.subtract), ("DI", "DR2", ALU.add)]
                out = []
                for i, (a, b, op) in enumerate(spec):
                    o = pool.tile([P, Wb], BF, **_nt(f"{tagp}o{i}"))
                    nc.vector.tensor_tensor(o[:], t_[a][:], t_[b][:], op)
                    out.append(o)
                return out[:4], out[4:]

            def einfft_block(last=False):
                KL = 256          # local k1 width
                with tc.tile_pool(name="ef", bufs=1) as ef:
                    ar3_in = [dram.tile([DIM // 2, L], BF, **_nt(f"ar3i{c}"))
                              for c in range(2)]
                    ar3_out = [dram.tile([DIM // 2, L], BF,
                                         **_nt(f"ar3o{c}")) for c in range(2)]
                    Xre = [ef.tile([P, KL], BF, **_nt(f"Xre{k}"))
                           for k in range(NCH)]
                    Xim = [ef.tile([P, KL], BF, **_nt(f"Xim{k}"))
                           for k in range(NCH)]
                    with tc.tile_pool(name="efa", bufs=1) as efa:
                        xn2 = layer_norm(n2_w, n2_b, efa, "xn2")
                        xnT = [efa.tile([P, DIM], BF, **_nt(f"xnT{t}"))
                               for t in range(8)]
                        with tc.tile_pool(name="psF", bufs=1,
                                          space="PSUM") as psF:
                            for t in range(8):
                                for k in range(NCH):
                                    pt = psF.tile([P, P], BF, **_nt("ptp"),
                                                  bufs=2)
                                    nc.tensor.transpose(
                                        pt[:], xn2[k][:, t * P:(t + 1) * P],
                                        ident_bf[:])
                                    nc.vector.tensor_copy(
                                        xnT[t][:, k * P:(k + 1) * P], pt[:])
                            for k in range(NCH):
                                pre = psF.tile([P, KL], F32, **_nt("pfr"),
                                               bufs=2)
                                pim = psF.tile([P, KL], F32, **_nt("pfi"),
                                               bufs=2)
                                for t in range(8):
                                    lhs = xnT[t][:, k * P:(k + 1) * P]
                                    nc.tensor.matmul(pre[:], lhs, CdF[t][:],
                                                     start=(t == 0),
                                                     stop=(t == 7))
                                    nc.tensor.matmul(pim[:], lhs, SdF[t][:],
                                                     start=(t == 0),
                                                     stop=(t == 7))
                                nc.vector.tensor_copy(Xre[k][:], pre[:])
                                nc.vector.tensor_scalar_mul(Xim[k][:],
                                                            pim[:], -1.0)

                    Xf_re, Xf_im = bfly(ef, Xre + Xim, "ff", KL)

                    r1 = [ef.tile([P, KL], BF, **_nt(f"r1_{b}"))
                          for b in range(NB)]
                    i1 = [ef.tile([P, KL], BF, **_nt(f"i1_{b}"))
                          for b in range(NB)]
                    with tc.tile_pool(name="psL1", bufs=2,
                                      space="PSUM") as psL1:
                        for b in range(NB):
                            pr = psL1.tile([P, KL], F32, **_nt("pl1r"))
                            nc.tensor.matmul(pr[:], w1r[b][:], Xf_re[b][:],
                                             start=True, stop=False)
                            nc.tensor.matmul(pr[:], w1in[b][:], Xf_im[b][:],
                                             start=False, stop=True)
                            nc.scalar.activation(r1[b][:], pr[:], AF.Relu,
                                                 bias=cb1r[b][:])
                            pi = psL1.tile([P, KL], F32, **_nt("pl1i"))
                            nc.tensor.matmul(pi[:], w1i[b][:], Xf_re[b][:],
                                             start=True, stop=False)
                            nc.tensor.matmul(pi[:], w1r[b][:], Xf_im[b][:],
                                             start=False, stop=True)
                            nc.scalar.activation(i1[b][:], pi[:], AF.Relu,
                                                 bias=cb1i[b][:])

                    zre = [None] * NB
                    zimN = [None] * NB
                    with tc.tile_pool(name="psL2", bufs=2,
                                      space="PSUM") as psL2:
                        for b in range(NB):
                            pzr = psL2.tile([P, KL], F32, **_nt("pl2r"))
                            nc.tensor.matmul(pzr[:], w2r[b][:], r1[b][:],
                                             start=True, stop=False)
                            nc.tensor.matmul(pzr[:], w2in[b][:], i1[b][:],
                                             start=False, stop=True)
                            a1 = ef.tile([P, KL], BF, **_nt("ss"), bufs=4)
                            nc.scalar.activation(a1[:], pzr[:], AF.Relu,
                                                 scale=0.5, bias=ssb[b][0][:])
                            a2 = ef.tile([P, KL], BF, **_nt("ss"), bufs=4)
                            nc.scalar.activation(a2[:], pzr[:], AF.Relu,
                                                 scale=-0.5,
                                                 bias=ssb[b][1][:])
                            zr = ef.tile([P, KL], BF, name=f"zre{b}",
                                         tag=f"Xre{b}")
                            nc.vector.tensor_tensor(zr[:], a1[:], a2[:],
                                                    ALU.subtract)
                            zre[b] = zr
                            pzi = psL2.tile([P, KL], F32, **_nt("pl2i"))
                            nc.tensor.matmul(pzi[:], w2i[b][:], r1[b][:],
                                             start=True, stop=False)
                            nc.tensor.matmul(pzi[:], w2r[b][:], i1[b][:],
                                             start=False, stop=True)
                            b1 = ef.tile([P, KL], BF, **_nt("ss"), bufs=4)
                            nc.scalar.activation(b1[:], pzi[:], AF.Relu,
                                                 scale=0.5, bias=ssb[b][2][:])
                            b2 = ef.tile([P, KL], BF, **_nt("ss"), bufs=4)
                            nc.scalar.activation(b2[:], pzi[:], AF.Relu,
                                                 scale=-0.5,
                                                 bias=ssb[b][3][:])
                            zi = ef.tile([P, KL], BF, name=f"zimN{b}",
                                         tag=f"Xim{b}")
                            nc.vector.tensor_tensor(zi[:], b2[:], b1[:],
                                                    ALU.subtract)
                            zimN[b] = zi

                    zz_re, zz_iN = bfly(ef, zre + zimN, "ff", KL)

                    with tc.tile_pool(name="psI", bufs=2,
                                      space="PSUM") as psI:
                        for b in range(NB):
                            zTr = ef.tile([P, KL], BF, **_nt("zzTr"),
                                          bufs=2)
                            zTi = ef.tile([P, KL], BF, **_nt("zzTi"),
                                          bufs=2)
                            for c in range(2):
                                pt = psI.tile([P, P], BF, **_nt("ptp2"))
                                nc.tensor.transpose(
                                    pt[:], zz_re[b][:, c * P:(c + 1) * P],
                                    ident_bf[:])
                                nc.vector.tensor_copy(
                                    zTr[:, c * P:(c + 1) * P], pt[:])
                                pt2 = psI.tile([P, P], BF, **_nt("ptp3"))
                                nc.tensor.transpose(
                                    pt2[:], zz_iN[b][:, c * P:(c + 1) * P],
                                    ident_bf[:])
                                nc.vector.tensor_copy(
                                    zTi[:, c * P:(c + 1) * P], pt2[:])
                            for h in range(2):
                                hs = slice(h * 512, (h + 1) * 512)
                                pout = psI.tile([P, 512], F32,
                                                **_nt("pidft"))
                                for c in range(2):
                                    nc.tensor.matmul(
                                        pout[:], zTr[:, c * P:(c + 1) * P],
                                        CdI[c][:, hs], start=(c == 0),
                                        stop=False)
                                    nc.tensor.matmul(
                                        pout[:], zTi[:, c * P:(c + 1) * P],
                                        SdI[c][:, hs], start=False,
                                        stop=(c == 1))
                                ob = ef.tile([P, 512], BF, **_nt("eob"),
                                             bufs=3)
                                nc.vector.tensor_copy(ob[:], pout[:])
                                if last:
                                    nc.sync.dma_start(
                                        xP_d[b * P:(b + 1) * P, hs], ob[:])
                                else:
                                    c, rr = divmod(b, 2)
                                    nc.sync.dma_start(
                                        ar3_in[c][rr * P:(rr + 1) * P, hs],
                                        ob[:])
                            if not last and b % 2 == 1:
                                c = b // 2
                                nc.gpsimd.collective_compute(
                                    "AllReduce", ALU.add,
                                    replica_groups=RG,
                                    ins=[ar3_in[c].opt()],
                                    outs=[ar3_out[c].opt()])

                    if not last:
                        for k in range(NCH):
                            c, rr = divmod(k, 2)
                            eo = ef.tile([P, L], BF, **_nt("eo"), bufs=2)
                            nc.sync.dma_start(
                                eo[:], ar3_out[c][rr * P:(rr + 1) * P, :])
                            nc.vector.tensor_tensor(x_res[k][:],
                                                    x_res[k][:], eo[:],
                                                    ALU.add)

            for blk in range(BLOCKS):
                mamba_block(blk)
                if blk == BLOCKS - 1:
                    for k in range(NCH):
                        nc.sync.dma_start(xO_d[k * P:(k + 1) * P, :],
                                          x_res[k][:])
                einfft_block(last=(blk == BLOCKS - 1))

    nc.compile()
    return nc


# --------------------------------------------------------------------------

def _make_inmaps(inputs):
    f32 = np.float32
    x = np.asarray(inputs["x"], f32)
    in_proj_w = np.asarray(inputs["in_proj_w"], f32)
    conv_w = np.asarray(inputs["conv_w"], f32)
    conv_b = np.asarray(inputs["conv_b"], f32)
    x_proj_w = np.asarray(inputs["x_proj_w"], f32)
    dt_proj_w = np.asarray(inputs["dt_proj_w"], f32)
    dt_proj_b = np.asarray(inputs["dt_proj_b"], f32)
    A_log = np.asarray(inputs["A_log"], f32)
    Dvec = np.asarray(inputs["D"], f32)
    out_proj_w = np.asarray(inputs["out_proj_w"], f32)
    ln_w = np.asarray(inputs["ln_w"], f32)
    ln_b = np.asarray(inputs["ln_b"], f32)
    n2_w = np.asarray(inputs["norm2_w"], f32)
    n2_b = np.asarray(inputs["norm2_b"], f32)
    cw1 = np.asarray(inputs["cw1"], f32)
    cw2 = np.asarray(inputs["cw2"], f32)
    cb1 = np.asarray(inputs["cb1"], f32)
    cb2 = np.asarray(inputs["cb2"], f32)

    n = np.arange(L, dtype=np.float64)
    ang = 2.0 * np.pi * np.outer(n, n) / L
    Cdft = (np.cos(ang) / np.sqrt(L)).astype(BF16)
    Sdft = (np.sin(ang) / np.sqrt(L)).astype(BF16)
    CdF = [np.ascontiguousarray(Cdft[:, r * 256:(r + 1) * 256])
           for r in range(GROUP)]
    SdF = [np.ascontiguousarray(Sdft[:, r * 256:(r + 1) * 256])
           for r in range(GROUP)]
    CdI = [np.ascontiguousarray(Cdft[r * 256:(r + 1) * 256, :])
           for r in range(GROUP)]
    SdI = [np.ascontiguousarray(Sdft[r * 256:(r + 1) * 256, :])
           for r in range(GROUP)]

    ssb = np.stack([
        (cb2[0] - LAM) / 2, (-cb2[0] - LAM) / 2,
        (cb2[1] - LAM) / 2, (-cb2[1] - LAM) / 2,
    ], axis=1)[:, :, :, None]

    in_maps = []
    for core in range(N_CORES):
        g, r = divmod(core, GROUP)
        lo, hi = r * DIL, (r + 1) * DIL
        # per-core channel order: own d_inner slice first, then the rest
        perm = np.r_[lo:hi, 0:lo, hi:DI]
        conv_wp = conv_w[perm, 0, :]
        conv_diag = np.zeros((NMT * DC, P, P), f32)
        for mt in range(NMT):
            for q in range(DC):
                np.fill_diagonal(conv_diag[mt * DC + q],
                                 conv_wp[mt * P:(mt + 1) * P, q])
        diag_D = np.zeros((NDT, P, P), f32)
        for j in range(NDT):
            np.fill_diagonal(diag_D[j], Dvec[lo + j * P: lo + (j + 1) * P])
        m = {
            "xT": np.ascontiguousarray(x[g].T).astype(BF16),
            "w_in": np.ascontiguousarray(
                np.concatenate([in_proj_w[perm],
                                in_proj_w[DI + lo:DI + hi]], 0).T
            ).astype(BF16),
            "conv_diag": conv_diag.astype(BF16),
            "conv_b": np.ascontiguousarray(conv_b[perm][:, None]),
            "w_xp": np.ascontiguousarray(x_proj_w[:, perm].T).astype(BF16),
            "w_dt": np.ascontiguousarray(dt_proj_w[lo:hi].T).astype(BF16),
            "dt_b": np.ascontiguousarray(dt_proj_b[lo:hi][:, None]),
            "A": np.ascontiguousarray(-np.exp(A_log[lo:hi])),
            "diag_D": diag_D.astype(BF16),
            "w_out": np.ascontiguousarray(
                out_proj_w[:, lo:hi].T).astype(BF16),
            "ln_w": np.ascontiguousarray(ln_w[:, None]),
            "ln_b": np.ascontiguousarray(ln_b[:, None]),
            "n2_w": np.ascontiguousarray(n2_w[:, None]),
            "n2_b": np.ascontiguousarray(n2_b[:, None]),
            "CdF": CdF[r], "SdF": SdF[r],
            "CdI": CdI[r], "SdI": SdI[r],
            "w1r": (0.5 * cw1[0]).astype(BF16),
            "w1i": (0.5 * cw1[1]).astype(BF16),
            "w1in": (-0.5 * cw1[1]).astype(BF16),
            "w2r": cw2[0].astype(BF16),
            "w2i": cw2[1].astype(BF16),
            "w2in": (-cw2[1]).astype(BF16),
            "cb1r": np.ascontiguousarray(cb1[0][:, :, None]),
            "cb1i": np.ascontiguousarray(cb1[1][:, :, None]),
            "ssb": np.ascontiguousarray(ssb, f32),
            "ident": np.eye(P, dtype=f32),
        }
        in_maps.append(m)
    return in_maps


def kernel(**inputs):
    global _COMPILED
    from concourse.bass_utils import run_bass_kernel_spmd
    if _COMPILED is None:
        _COMPILED = _build_program()
    in_maps = _make_inmaps(inputs)
    res = run_bass_kernel_spmd(_COMPILED, in_maps,
                               core_ids=list(range(N_CORES)))
    outs = []
    for g in range(2):
        x = res.results[g * GROUP]["xO"].astype(np.float32)
        for r in range(GROUP):
            x = x + res.results[g * GROUP + r]["xP"].astype(np.float32)
        outs.append(x.T)
    return np.ascontiguousarray(np.stack(outs).astype(np.float32))


# revision 30
# speedup vs baseline: 1.0229x; 1.0048x over previous
"""Trainium2 Bass kernel for nn_Block_mamba (SiMBA-style block: Mamba + EinFFT).

8 NeuronCores = 2 batch groups x 4-way shard of d_inner (256 ch/core).
Design notes (DVE is the bottleneck; hw scans are a fixed ~2.1ns/elem
cost, dtype independent; concurrent GpSimd work degrades DVE ~4x so the
scan loop is vector+scalar+PE only):
 - in_proj/conv replicated per core (no AllGather); depthwise conv runs on
   the PE as diagonal matmuls over a left-padded input.
 - scan loop per s: one [1,L]->[128,L] DMA broadcast each for B_s/C_s
   (shared by both local d-tiles), wide [128,2048] dB/p multiplies via a
   stride-0 repeat AP on the broadcast tile, two hw scans (dA fp32 from
   the scalar engine, dB/h/p bf16), and PE identity-matmul accumulation
   of p into PSUM (fp32, exact); D*xm is folded in via a diagonal-D
   matmul that opens the accumulation group.
 - PE p-state: prologue matmul streams are emitted gap-free (psA bufs=3,
   conv(mt+1) ahead of xproj(mt)) so the PE ramps to full clock.
 - AllReduces are split in two [256,L] chunks on separate DRAM tiles so
   transfer overlaps producer/consumer work.
 - residual stream bf16; LN uses PE for stats/broadcast, tensor_scalar 4x.

kernel(**inputs): full unsharded inputs -> full (2, 1024, 512) output.
"""

import numpy as np
import ml_dtypes

DIM = 512
NB = 4
BS = 128
DS = 64
DC = 4
DI = 1024
DTR = 32
BLOCKS = 2
LAM = 0.01
L = 1024

N_CORES = 8
GROUP = 4
DIL = DI // GROUP       # 256
P = 128
NDT = DIL // P          # 2
NCH = DIM // P          # 4
NMT = DI // P           # 8 xm tiles (replicated)
W = 2 * L               # wide free dim (both d-tiles packed)

BF16 = ml_dtypes.bfloat16

_COMPILED = None


def _nt(s):
    return {"name": s, "tag": s}


def _build_program():
    import contextlib
    import concourse.bacc as bacc
    import concourse.mybir as mybir
    import concourse.tile as tile

    F32 = mybir.dt.float32
    BF = mybir.dt.bfloat16
    AF = mybir.ActivationFunctionType
    ALU = mybir.AluOpType

    nc = bacc.Bacc("TRN2", target_bir_lowering=False, debug=False,
                   num_devices=N_CORES)

    _eps = nc.alloc_sbuf_tensor("const-float32-eps", [128, 1], F32)
    nc.gpsimd.memset(_eps.ap(), 1e-5)
    nc.const_aps.aps[(F32, 1e-5)] = _eps.ap()
    nc.all_engine_barrier()

    def din(name, shape, dt=F32):
        return nc.dram_tensor(name, shape, dt, kind="ExternalInput")

    xT_d = din("xT", [DIM, L], BF)
    w_in_d = din("w_in", [DIM, DI + DIL], BF)
    conv_diag_d = din("conv_diag", [NMT * DC, P, P], BF)
    conv_b_d = din("conv_b", [DI, 1])
    w_xp_d = din("w_xp", [DI, DTR + 2 * DS], BF)
    w_dt_d = din("w_dt", [DTR, DIL], BF)
    dt_b_d = din("dt_b", [DIL, 1])
    A_d = din("A", [DIL, DS])
    diag_D_d = din("diag_D", [NDT, P, P], BF)
    w_out_d = din("w_out", [DIL, DIM], BF)
    ln_w_d = din("ln_w", [DIM, 1])
    ln_b_d = din("ln_b", [DIM, 1])
    n2_w_d = din("n2_w", [DIM, 1])
    n2_b_d = din("n2_b", [DIM, 1])
    CdF_d = din("CdF", [L, 256], BF)    # C[:, k1_loc] forward
    SdF_d = din("SdF", [L, 256], BF)
    CdI_d = din("CdI", [256, L], BF)    # C[k1_loc, :] inverse
    SdI_d = din("SdI", [256, L], BF)
    w1r_d = din("w1r", [NB, BS, BS], BF)
    w1i_d = din("w1i", [NB, BS, BS], BF)
    w1in_d = din("w1in", [NB, BS, BS], BF)
    w2r_d = din("w2r", [NB, BS, BS], BF)
    w2i_d = din("w2i", [NB, BS, BS], BF)
    w2in_d = din("w2in", [NB, BS, BS], BF)
    cb1r_d = din("cb1r", [NB, BS, 1])
    cb1i_d = din("cb1i", [NB, BS, 1])
    ssb_d = din("ssb", [NB, 4, BS, 1])
    ident_d = din("ident", [P, P])
    xO_d = nc.dram_tensor("xO", [DIM, L], BF, kind="ExternalOutput")
    xP_d = nc.dram_tensor("xP", [DIM, L], BF, kind="ExternalOutput")

    RG = [[0, 1, 2, 3], [4, 5, 6, 7]]

    with tile.TileContext(nc) as tc:
        stack = contextlib.ExitStack()
        with stack:
            wp = stack.enter_context(tc.tile_pool(name="wp", bufs=1))
            ap = stack.enter_context(tc.tile_pool(name="ap", bufs=1))
            lnp = stack.enter_context(tc.tile_pool(name="lnp", bufs=1))
            dram = stack.enter_context(
                tc.tile_pool(name="dram", bufs=1, space="DRAM"))

            # residual stream (bf16)
            x_res = [ap.tile([P, L], BF, **_nt(f"xres{k}"))
                     for k in range(NCH)]
            for k in range(NCH):
                nc.sync.dma_start(x_res[k][:], xT_d[k * P:(k + 1) * P, :])

            def wtile(src, shape, dt=F32, name=None, tag=None):
                t = wp.tile(shape, dt, name=name, tag=tag)
                nc.sync.dma_start(t[:], src)
                return t

            # critical-path weights first
            ln_w = [wtile(ln_w_d[k * P:(k + 1) * P, :], [P, 1],
                          **_nt(f"lnw{k}")) for k in range(NCH)]
            ln_b = [wtile(ln_b_d[k * P:(k + 1) * P, :], [P, 1],
                          **_nt(f"lnb{k}")) for k in range(NCH)]
            w_in = [wtile(w_in_d[k * P:(k + 1) * P, :], [P, DI + DIL], BF,
                          **_nt(f"w_in{k}")) for k in range(NCH)]
            conv_diag = [wtile(conv_diag_d[i], [P, P], BF,
                               **_nt(f"cvd{i}")) for i in range(NMT * DC)]
            conv_b = [wtile(conv_b_d[m * P:(m + 1) * P, :], [P, 1],
                            **_nt(f"convb{m}")) for m in range(NMT)]
            w_xp = [wtile(w_xp_d[k * P:(k + 1) * P, :], [P, DTR + 2 * DS],
                          BF, **_nt(f"w_xp{k}")) for k in range(NMT)]
            w_dt = wtile(w_dt_d[:], [DTR, DIL], BF, **_nt("w_dt"))
            dt_b = [wtile(dt_b_d[j * P:(j + 1) * P, :], [P, 1],
                          **_nt(f"dtb{j}")) for j in range(NDT)]
            A_t = [wtile(A_d[j * P:(j + 1) * P, :], [P, DS],
                         **_nt(f"A{j}")) for j in range(NDT)]
            diag_D = [wtile(diag_D_d[j], [P, P], BF, **_nt(f"dD{j}"))
                      for j in range(NDT)]
            w_out = [wtile(w_out_d[j * P:(j + 1) * P, :], [P, DIM], BF,
                           **_nt(f"w_out{j}")) for j in range(NDT)]
            n2_w = [wtile(n2_w_d[k * P:(k + 1) * P, :], [P, 1],
                          **_nt(f"n2w{k}")) for k in range(NCH)]
            n2_b = [wtile(n2_b_d[k * P:(k + 1) * P, :], [P, 1],
                          **_nt(f"n2b{k}")) for k in range(NCH)]
            ident = wtile(ident_d[:], [P, P], **_nt("ident"))
            ident_bf = wp.tile([P, P], BF, **_nt("ident_bf"))
            nc.vector.tensor_copy(ident_bf[:], ident[:])
            # einfft weights (needed latest)
            CdF = [wtile(CdF_d[t * P:(t + 1) * P, :], [P, 256], BF,
                         **_nt(f"CdF{t}")) for t in range(8)]
            SdF = [wtile(SdF_d[t * P:(t + 1) * P, :], [P, 256], BF,
                         **_nt(f"SdF{t}")) for t in range(8)]
            CdI = [wtile(CdI_d[c * P:(c + 1) * P, :], [P, L], BF,
                         **_nt(f"CdI{c}")) for c in range(2)]
            SdI = [wtile(SdI_d[c * P:(c + 1) * P, :], [P, L], BF,
                         **_nt(f"SdI{c}")) for c in range(2)]
            w1r = [wtile(w1r_d[b], [BS, BS], BF, **_nt(f"w1r{b}"))
                   for b in range(NB)]
            w1i = [wtile(w1i_d[b], [BS, BS], BF, **_nt(f"w1i{b}"))
                   for b in range(NB)]
            w1in = [wtile(w1in_d[b], [BS, BS], BF, **_nt(f"w1in{b}"))
                    for b in range(NB)]
            w2r = [wtile(w2r_d[b], [BS, BS], BF, **_nt(f"w2r{b}"))
                   for b in range(NB)]
            w2i = [wtile(w2i_d[b], [BS, BS], BF, **_nt(f"w2i{b}"))
                   for b in range(NB)]
            w2in = [wtile(w2in_d[b], [BS, BS], BF, **_nt(f"w2in{b}"))
                    for b in range(NB)]
            cb1r = [wtile(cb1r_d[b], [BS, 1], **_nt(f"cb1r{b}"))
                    for b in range(NB)]
            cb1i = [wtile(cb1i_d[b], [BS, 1], **_nt(f"cb1i{b}"))
                    for b in range(NB)]
            ssb = [[wtile(ssb_d[b, jj], [BS, 1], **_nt(f"ssb{b}_{jj}"))
                    for jj in range(4)] for b in range(NB)]

            ones_k1 = wp.tile([1, P], BF, **_nt("ones_k1"))
            nc.vector.memset(ones_k1[:], 1.0)
            ones_m1 = wp.tile([P, 1], BF, **_nt("ones_m1"))
            nc.vector.memset(ones_m1[:], 1.0)

            # ----------------------------------------------------------
            def layer_norm(w_aps, b_aps, pool, out_tag):
                """bf16 LN over partition dim (d) of x_res; returns bf16."""
                with tc.tile_pool(name="psln", bufs=1, space="PSUM") as psl:
                    pm = psl.tile([1, L], F32, **_nt("ln_pm"))
                    for k in range(NCH):
                        for h in range(2):
                            hs = slice(h * 512, (h + 1) * 512)
                            nc.tensor.matmul(pm[:, hs], ones_m1[:],
                                             x_res[k][:, hs],
                                             start=(k == 0),
                                             stop=(k == NCH - 1))
                    psq = psl.tile([1, L], F32, **_nt("ln_psq"))
                    for k in range(NCH):
                        x2 = lnp.tile([P, L], BF, **_nt("ln_x2"), bufs=2)
                        nc.scalar.activation(x2[:], x_res[k][:], AF.Square)
                        for h in range(2):
                            hs = slice(h * 512, (h + 1) * 512)
                            nc.tensor.matmul(psq[:, hs], ones_m1[:],
                                             x2[:, hs], start=(k == 0),
                                             stop=(k == NCH - 1))
                    nm = lnp.tile([1, L], BF, **_nt("ln_nm"))
                    nc.scalar.mul(nm[:], pm[:], -1.0 / DIM)
                    msq = lnp.tile([1, L], F32, **_nt("ln_msq"))
                    nc.scalar.activation(msq[:], pm[:], AF.Square,
                                         scale=1.0 / DIM)
                    ch = lnp.tile([1, L], F32, **_nt("ln_ch"))
                    nc.vector.scalar_tensor_tensor(
                        ch[:], psq[:], 1.0 / DIM, msq[:],
                        ALU.mult, ALU.subtract)
                    inv = lnp.tile([1, L], BF, **_nt("ln_inv"))
                    nc.scalar.activation(inv[:], ch[:],
                                         AF.Abs_reciprocal_sqrt, bias=1e-5)
                    nm_ps = psl.tile([P, L], F32, **_nt("ln_nmps"))
                    iv_ps = psl.tile([P, L], F32, **_nt("ln_ivps"))
                    for h in range(2):
                        hs = slice(h * 512, (h + 1) * 512)
                        nc.tensor.matmul(nm_ps[:, hs], ones_k1[:],
                                         nm[:, hs], start=True, stop=True)
                        nc.tensor.matmul(iv_ps[:, hs], ones_k1[:],
                                         inv[:, hs], start=True, stop=True)
                    nm_bc = lnp.tile([P, L], BF, **_nt("ln_nmbc"))
                    nc.scalar.copy(nm_bc[:], nm_ps[:])
                    iv_bc = lnp.tile([P, L], BF, **_nt("ln_ivbc"))
                    nc.scalar.copy(iv_bc[:], iv_ps[:])
                    outs = []
                    for k in range(NCH):
                        t1 = lnp.tile([P, L], BF, **_nt("ln_t1"), bufs=3)
                        nc.vector.tensor_tensor(t1[:], x_res[k][:],
                                                nm_bc[:], ALU.add)
                        t2 = lnp.tile([P, L], BF, **_nt("ln_t2"), bufs=3)
                        nc.vector.tensor_tensor(t2[:], t1[:], iv_bc[:],
                                                ALU.mult)
                        o = pool.tile([P, L], BF, **_nt(f"{out_tag}{k}"))
                        nc.vector.tensor_scalar(o[:], t2[:], w_aps[k][:],
                                                b_aps[k][:], ALU.mult,
                                                ALU.add)
                        outs.append(o)
                    return outs

            # ----------------------------------------------------------
            def mamba_block(blk):
                with tc.tile_pool(name="mb", bufs=1) as mb:
                    ar2_in = [dram.tile([DIM // 2, L], BF, **_nt(f"ar2i{c}"))
                              for c in range(2)]
                    ar2_out = [dram.tile([DIM // 2, L], BF,
                                         **_nt(f"ar2o{c}")) for c in range(2)]
                    projbd = dram.tile([2 * DS, L], BF, **_nt("projbd"))

                    xmc_wide = mb.tile([P, W], BF, **_nt("xmc_wide"))
                    szs_wide = mb.tile([P, W], BF, **_nt("szs_wide"))
                    dt_wide = mb.tile([P, W], F32, **_nt("dt_wide"))
                    du_wide = mb.tile([P, W], BF, **_nt("du_wide"))
                    proj_dt = mb.tile([DTR, L], BF, **_nt("proj_dt"))
                    y2 = [mb.tile([P, L], BF, **_nt(f"y2_{j}"))
                          for j in range(NDT)]

                    with tc.tile_pool(name="mpre", bufs=1) as mpre:
                        xn = layer_norm(ln_w, ln_b, mpre, "xn")
                        # --- in_proj (all 8 xm tiles + own 2 z) pipelined
                        # with the PE depthwise conv + x_proj accumulation.
                        LP = L + DC - 1
                        xm_sb = [mpre.tile([P, LP], BF, **_nt(f"xmsb{m}"))
                                 for m in range(NMT)]
                        for m in range(NMT):
                            nc.gpsimd.memset(xm_sb[m][:, 0:DC - 1], 0.0)
                        with tc.tile_pool(name="psA", bufs=3,
                                          space="PSUM") as psA:
                            for mt in range(NMT + NDT):
                                pxz = psA.tile([P, L], F32, **_nt("pxz"))
                                for k in range(NCH):
                                    lhs = w_in[k][:, mt * P:(mt + 1) * P]
                                    for h in range(2):
                                        hs = slice(h * 512, (h + 1) * 512)
                                        nc.tensor.matmul(
                                            pxz[:, hs], lhs, xn[k][:, hs],
                                            start=(k == 0),
                                            stop=(k == NCH - 1))
                                if mt < NMT:
                                    nc.vector.tensor_copy(
                                        xm_sb[mt][:, DC - 1:LP], pxz[:])
                                else:
                                    j = mt - NMT
                                    nc.scalar.activation(
                                        szs_wide[:, j * L:(j + 1) * L],
                                        pxz[:], AF.Silu)
                        with tc.tile_pool(name="psC", bufs=2,
                                          space="PSUM") as psC, \
                             tc.tile_pool(name="psP", bufs=1,
                                          space="PSUM") as psP:
                            pp1 = psP.tile([P, L], F32, **_nt("pp1"))
                            pp2 = psP.tile([32, L], F32, **_nt("pp2"))
                            xmc = [None] * NMT

                            def emit_conv(mt):
                                psc = psC.tile([P, L], F32, **_nt("psc"))
                                for q in range(DC):
                                    dg = conv_diag[mt * DC + q]
                                    for h in range(2):
                                        hs = slice(h * 512, (h + 1) * 512)
                                        nc.tensor.matmul(
                                            psc[:, hs], dg[:],
                                            xm_sb[mt][:, q + h * 512:
                                                       q + (h + 1) * 512],
                                            start=(q == 0),
                                            stop=(q == DC - 1))
                                # own-first channel order: own d-tiles are
                                # mt 0..NDT-1.
                                if mt < NDT:
                                    nc.scalar.activation(
                                        xmc_wide[:, mt * L:(mt + 1) * L],
                                        psc[:], AF.Silu,
                                        bias=conv_b[mt][:])
                                    xmc[mt] = (xmc_wide,
                                               slice(mt * L, (mt + 1) * L))
                                else:
                                    t = mpre.tile([P, L], BF,
                                                  **_nt("xmct"), bufs=4)
                                    nc.scalar.activation(t[:], psc[:],
                                                         AF.Silu,
                                                         bias=conv_b[mt][:])
                                    xmc[mt] = (t, slice(0, L))

                            def emit_xproj(mt):
                                src, sl = xmc[mt]
                                for h in range(2):
                                    hs = slice(h * 512, (h + 1) * 512)
                                    rhs = src[:, sl.start + h * 512:
                                              sl.start + (h + 1) * 512]
                                    nc.tensor.matmul(
                                        pp1[:, hs], w_xp[mt][:, 0:P], rhs,
                                        start=(mt == 0),
                                        stop=(mt == NMT - 1))
                                    nc.tensor.matmul(
                                        pp2[:, hs], w_xp[mt][:, P:160], rhs,
                                        start=(mt == 0),
                                        stop=(mt == NMT - 1))

                            # emit conv(mt+1) before xproj(mt) so the PE
                            # never stalls on the silu between them
                            emit_conv(0)
                            for mt in range(1, NMT):
                                emit_conv(mt)
                                emit_xproj(mt - 1)
                            emit_xproj(NMT - 1)
                            # extract dt-proj input + B/C rows
                            nc.vector.tensor_copy(proj_dt[:],
                                                  pp1[0:DTR, :])
                            pjA = mpre.tile([P, L], BF, **_nt("pjA"))
                            nc.vector.tensor_copy(pjA[:], pp1[:])
                            pjB = mpre.tile([32, L], BF, **_nt("pjB"))
                            nc.vector.tensor_copy(pjB[:], pp2[:])
                            nc.sync.dma_start(projbd[0:96, :],
                                              pjA[DTR:P, :])
                            nc.sync.dma_start(projbd[96:128, :], pjB[:])
                        # --- dt_proj + softplus into dt_wide halves ---
                        with tc.tile_pool(name="psD", bufs=2,
                                          space="PSUM") as psD:
                            for j in range(NDT):
                                pdt = psD.tile([P, L], F32, **_nt("pdt"))
                                for h in range(2):
                                    hs = slice(h * 512, (h + 1) * 512)
                                    nc.tensor.matmul(
                                        pdt[:, hs],
                                        w_dt[:, j * P:(j + 1) * P],
                                        proj_dt[:, hs], start=True,
                                        stop=True)
                                dtj = dt_wide[:, j * L:(j + 1) * L]
                                nc.scalar.activation(dtj, pdt[:], AF.Exp,
                                                     bias=dt_b[j][:])
                                nc.scalar.activation(dtj, dtj, AF.Ln,
                                                     bias=1.0)
                        nc.vector.tensor_tensor(du_wide[:], dt_wide[:],
                                                xmc_wide[:], ALU.mult)

                    # ------------- scan loop --------------------------
                    with tc.tile_pool(name="msc", bufs=1) as msc, \
                         tc.tile_pool(name="psY", bufs=1,
                                      space="PSUM") as psY:
                        psum_y = [psY.tile([P, L], F32, **_nt(f"py{j}"))
                                  for j in range(NDT)]
                        # init: psum_y[j] = diag(D_j) @ xmc_j
                        for j in range(NDT):
                            for h in range(2):
                                hs = slice(h * 512, (h + 1) * 512)
                                nc.tensor.matmul(
                                    psum_y[j][:, hs], diag_D[j][:],
                                    xmc_wide[:, j * L + h * 512:
                                             j * L + (h + 1) * 512],
                                    start=True, stop=False)
                        def rep3(t):
                            return t[:].unsqueeze(1).to_broadcast(
                                (P, NDT, L))

                        def wide3(t):
                            return t[:].rearrange("p (a b) -> p a b",
                                                  a=NDT)

                        for s in range(DS):
                            # one broadcast per s, shared by both d-tiles
                            bB = msc.tile([P, L], BF, **_nt("bB"), bufs=6)
                            nc.sync.dma_start(
                                bB[:],
                                projbd[s:s + 1, :].to_broadcast((P, L)))
                            bC = msc.tile([P, L], BF, **_nt("bC"), bufs=6)
                            nc.sync.dma_start(
                                bC[:],
                                projbd[DS + s:DS + s + 1,
                                       :].to_broadcast((P, L)))
                            last = (s == DS - 1)
                            # wide dB for both d-tiles in one op (stride-0
                            # repeat of the broadcast row tile)
                            dB = msc.tile([P, W], BF, **_nt("dB"), bufs=3)
                            nc.vector.tensor_tensor(
                                wide3(dB), wide3(du_wide), rep3(bB),
                                ALU.mult)
                            h_t = msc.tile([P, W], BF, **_nt("h"), bufs=3)
                            for j in range(NDT):
                                js = slice(j * L, (j + 1) * L)
                                dA = msc.tile([P, L], F32, **_nt("dA"),
                                              bufs=4)
                                nc.scalar.activation(
                                    dA[:], dt_wide[:, js], AF.Exp,
                                    scale=A_t[j][:, s:s + 1])
                                nc.vector.tensor_tensor_scan(
                                    h_t[:, js], dA[:], dB[:, js], 0.0,
                                    ALU.mult, ALU.add)
                            p = msc.tile([P, W], BF, **_nt("p"), bufs=3)
                            nc.vector.tensor_tensor(
                                wide3(p), wide3(h_t), rep3(bC), ALU.mult)
                            for j in range(NDT):
                                for h in range(2):
                                    hs = slice(h * 512, (h + 1) * 512)
                                    nc.tensor.matmul(
                                        psum_y[j][:, hs], ident_bf[:],
                                        p[:, j * L + h * 512:
                                          j * L + (h + 1) * 512],
                                        start=False, stop=last)
                        # --- gate + out_proj ---
                        for j in range(NDT):
                            nc.vector.tensor_tensor(
                                y2[j][:], psum_y[j][:],
                                szs_wide[:, j * L:(j + 1) * L], ALU.mult)
                    with tc.tile_pool(name="mpost", bufs=1) as mpost, \
                         tc.tile_pool(name="psO", bufs=2,
                                      space="PSUM") as psO:
                        for mt in range(NCH):
                            po = psO.tile([P, L], F32, **_nt("pout"))
                            for h in range(2):
                                hs = slice(h * 512, (h + 1) * 512)
                                for j in range(NDT):
                                    nc.tensor.matmul(
                                        po[:, hs],
                                        w_out[j][:, mt * P:(mt + 1) * P],
                                        y2[j][:, hs], start=(j == 0),
                                        stop=(j == NDT - 1))
                            osb = mpost.tile([P, L], BF, **_nt("ar2sb"),
                                             bufs=2)
                            nc.scalar.copy(osb[:], po[:])
                            c, rr = divmod(mt, 2)
                            nc.sync.dma_start(
                                ar2_in[c][rr * P:(rr + 1) * P, :], osb[:])
                            if rr == 1:
                                nc.gpsimd.collective_compute(
                                    "AllReduce", ALU.add,
                                    replica_groups=RG,
                                    ins=[ar2_in[c].opt()],
                                    outs=[ar2_out[c].opt()])
                        for k in range(NCH):
                            c, rr = divmod(k, 2)
                            mo = mpost.tile([P, L], BF, **_nt("mo"),
                                            bufs=2)
                            nc.sync.dma_start(
                                mo[:], ar2_out[c][rr * P:(rr + 1) * P, :])
                            nc.vector.tensor_tensor(x_res[k][:],
                                                    x_res[k][:], mo[:],
                                                    ALU.add)

            # ----------------------------------------------------------
            def bfly(pool, pl, tagp, Wb=L):
                R, I = pl[:4], pl[4:]
                t_ = {}
                for nm, (a, b, op) in {
                    "SR": (R[0], R[2], ALU.add),
                    "DR": (R[0], R[2], ALU.subtract),
                    "SR2": (R[1], R[3], ALU.add),
                    "DR2": (R[1], R[3], ALU.subtract),
                    "SI": (I[0], I[2], ALU.add),
                    "DI": (I[0], I[2], ALU.subtract),
                    "SI2": (I[1], I[3], ALU.add),
                    "DI2": (I[1], I[3], ALU.subtract),
                }.items():
                    tt = pool.tile([P, Wb], BF, **_nt(f"{tagp}t_{nm}"))
                    nc.vector.tensor_tensor(tt[:], a[:], b[:], op)
                    t_[nm] = tt
                spec = [("SR", "SR2", ALU.add), ("DR", "DI2", ALU.add),
                        ("SR", "SR2", ALU.subtract),
                        ("DR", "DI2", ALU.subtract),
                        ("SI", "SI2", ALU.add), ("DI", "DR2", ALU.subtract),
                        ("SI", "SI2", ALU.subtract), ("DI", "DR2", ALU.add)]
                out = []
                for i, (a, b, op) in enumerate(spec):
                    o = pool.tile([P, Wb], BF, **_nt(f"{tagp}o{i}"))
                    nc.vector.tensor_tensor(o[:], t_[a][:], t_[b][:], op)
                    out.append(o)
                return out[:4], out[4:]

            def einfft_block(last=False):
                KL = 256          # local k1 width
                with tc.tile_pool(name="ef", bufs=1) as ef:
                    ar3_in = [dram.tile([DIM // 2, L], BF, **_nt(f"ar3i{c}"))
                              for c in range(2)]
                    ar3_out = [dram.tile([DIM // 2, L], BF,
                                         **_nt(f"ar3o{c}")) for c in range(2)]
                    Xre = [ef.tile([P, KL], BF, **_nt(f"Xre{k}"))
                           for k in range(NCH)]
                    Xim = [ef.tile([P, KL], BF, **_nt(f"Xim{k}"))
                           for k in range(NCH)]
                    with tc.tile_pool(name="efa", bufs=1) as efa:
                        xn2 = layer_norm(n2_w, n2_b, efa, "xn2")
                        xnT = [efa.tile([P, DIM], BF, **_nt(f"xnT{t}"))
                               for t in range(8)]
                        with tc.tile_pool(name="psF", bufs=1,
                                          space="PSUM") as psF:
                            for t in range(8):
                                for k in range(NCH):
                                    pt = psF.tile([P, P], BF, **_nt("ptp"),
                                                  bufs=2)
                                    nc.tensor.transpose(
                                        pt[:], xn2[k][:, t * P:(t + 1) * P],
                                        ident_bf[:])
                                    nc.vector.tensor_copy(
                                        xnT[t][:, k * P:(k + 1) * P], pt[:])
                            for k in range(NCH):
                                pre = psF.tile([P, KL], F32, **_nt("pfr"),
                                               bufs=2)
                                pim = psF.tile([P, KL], F32, **_nt("pfi"),
                                               bufs=2)
                                for t in range(8):
                                    lhs = xnT[t][:, k * P:(k + 1) * P]
                                    nc.tensor.matmul(pre[:], lhs, CdF[t][:],
                                                     start=(t == 0),
                                                     stop=(t == 7))
                                    nc.tensor.matmul(pim[:], lhs, SdF[t][:],
                                                     start=(t == 0),
                                                     stop=(t == 7))
                                nc.vector.tensor_copy(Xre[k][:], pre[:])
                                nc.vector.tensor_scalar_mul(Xim[k][:],
                                                            pim[:], -1.0)

                    Xf_re, Xf_im = bfly(ef, Xre + Xim, "ff", KL)

                    r1 = [ef.tile([P, KL], BF, **_nt(f"r1_{b}"))
                          for b in range(NB)]
                    i1 = [ef.tile([P, KL], BF, **_nt(f"i1_{b}"))
                          for b in range(NB)]
                    with tc.tile_pool(name="psL1", bufs=2,
                                      space="PSUM") as psL1:
                        for b in range(NB):
                            pr = psL1.tile([P, KL], F32, **_nt("pl1r"))
                            nc.tensor.matmul(pr[:], w1r[b][:], Xf_re[b][:],
                                             start=True, stop=False)
                            nc.tensor.matmul(pr[:], w1in[b][:], Xf_im[b][:],
                                             start=False, stop=True)
                            nc.scalar.activation(r1[b][:], pr[:], AF.Relu,
                                                 bias=cb1r[b][:])
                            pi = psL1.tile([P, KL], F32, **_nt("pl1i"))
                            nc.tensor.matmul(pi[:], w1i[b][:], Xf_re[b][:],
                                             start=True, stop=False)
                            nc.tensor.matmul(pi[:], w1r[b][:], Xf_im[b][:],
                                             start=False, stop=True)
                            nc.scalar.activation(i1[b][:], pi[:], AF.Relu,
                                                 bias=cb1i[b][:])

                    zre = [None] * NB
                    zimN = [None] * NB
                    with tc.tile_pool(name="psL2", bufs=2,
                                      space="PSUM") as psL2:
                        for b in range(NB):
                            pzr = psL2.tile([P, KL], F32, **_nt("pl2r"))
                            nc.tensor.matmul(pzr[:], w2r[b][:], r1[b][:],
                                             start=True, stop=False)
                            nc.tensor.matmul(pzr[:], w2in[b][:], i1[b][:],
                                             start=False, stop=True)
                            a1 = ef.tile([P, KL], BF, **_nt("ss"), bufs=4)
                            nc.scalar.activation(a1[:], pzr[:], AF.Relu,
                                                 scale=0.5, bias=ssb[b][0][:])
                            a2 = ef.tile([P, KL], BF, **_nt("ss"), bufs=4)
                            nc.scalar.activation(a2[:], pzr[:], AF.Relu,
                                                 scale=-0.5,
                                                 bias=ssb[b][1][:])
                            zr = ef.tile([P, KL], BF, name=f"zre{b}",
                                         tag=f"Xre{b}")
                            nc.vector.tensor_tensor(zr[:], a1[:], a2[:],
                                                    ALU.subtract)
                            zre[b] = zr
                            pzi = psL2.tile([P, KL], F32, **_nt("pl2i"))
                            nc.tensor.matmul(pzi[:], w2i[b][:], r1[b][:],
                                             start=True, stop=False)
                            nc.tensor.matmul(pzi[:], w2r[b][:], i1[b][:],
                                             start=False, stop=True)
                            b1 = ef.tile([P, KL], BF, **_nt("ss"), bufs=4)
                            nc.scalar.activation(b1[:], pzi[:], AF.Relu,
                                                 scale=0.5, bias=ssb[b][2][:])
                            b2 = ef.tile([P, KL], BF, **_nt("ss"), bufs=4)
                            nc.scalar.activation(b2[:], pzi[:], AF.Relu,
                                                 scale=-0.5,
                                                 bias=ssb[b][3][:])
                            zi = ef.tile([P, KL], BF, name=f"zimN{b}",
                                         tag=f"Xim{b}")
                            nc.vector.tensor_tensor(zi[:], b2[:], b1[:],
                                                    ALU.subtract)
                            zimN[b] = zi

                    zz_re, zz_iN = bfly(ef, zre + zimN, "ff", KL)

                    with tc.tile_pool(name="psI", bufs=2,
                                      space="PSUM") as psI:
                        for b in range(NB):
                            zTr = ef.tile([P, KL], BF, **_nt("zzTr"),
                                          bufs=2)
                            zTi = ef.tile([P, KL], BF, **_nt("zzTi"),
                                          bufs=2)
                            for c in range(2):
                                pt = psI.tile([P, P], BF, **_nt("ptp2"))
                                nc.tensor.transpose(
                                    pt[:], zz_re[b][:, c * P:(c + 1) * P],
                                    ident_bf[:])
                                nc.vector.tensor_copy(
                                    zTr[:, c * P:(c + 1) * P], pt[:])
                                pt2 = psI.tile([P, P], BF, **_nt("ptp3"))
                                nc.tensor.transpose(
                                    pt2[:], zz_iN[b][:, c * P:(c + 1) * P],
                                    ident_bf[:])
                                nc.vector.tensor_copy(
                                    zTi[:, c * P:(c + 1) * P], pt2[:])
                            for h in range(2):
                                hs = slice(h * 512, (h + 1) * 512)
                                pout = psI.tile([P, 512], F32,
                                                **_nt("pidft"))
                                for c in range(2):
                                    nc.tensor.matmul(
                                        pout[:], zTr[:, c * P:(c + 1) * P],
                                        CdI[c][:, hs], start=(c == 0),
                                        stop=False)
                                    nc.tensor.matmul(
                                        pout[:], zTi[:, c * P:(c + 1) * P],
                                        SdI[c][:, hs], start=False,
                                        stop=(c == 1))
                                ob = ef.tile([P, 512], BF, **_nt("eob"),
                                             bufs=3)
                                nc.vector.tensor_copy(ob[:], pout[:])
                                if last:
                                    nc.sync.dma_start(
                                        xP_d[b * P:(b + 1) * P, hs], ob[:])
                                else:
                                    c, rr = divmod(b, 2)
                                    nc.sync.dma_start(
                                        ar3_in[c][rr * P:(rr + 1) * P, hs],
                                        ob[:])
                            if not last and b % 2 == 1:
                                c = b // 2
                                nc.gpsimd.collective_compute(
                                    "AllReduce", ALU.add,
                                    replica_groups=RG,
                                    ins=[ar3_in[c].opt()],
                                    outs=[ar3_out[c].opt()])

                    if not last:
                        for k in range(NCH):
                            c, rr = divmod(k, 2)
                            eo = ef.tile([P, L], BF, **_nt("eo"), bufs=2)
                            nc.sync.dma_start(
                                eo[:], ar3_out[c][rr * P:(rr + 1) * P, :])
                            nc.vector.tensor_tensor(x_res[k][:],
                                                    x_res[k][:], eo[:],
                                                    ALU.add)

            for blk in range(BLOCKS):
                mamba_block(blk)
                if blk == BLOCKS - 1:
                    for k in range(NCH):
                        nc.sync.dma_start(xO_d[k * P:(k + 1) * P, :],
                                          x_res[k][:])
                einfft_block(last=(blk == BLOCKS - 1))

    nc.compile()
    return nc


# --------------------------------------------------------------------------

def _make_inmaps(inputs):
    f32 = np.float32
    x = np.asarray(inputs["x"], f32)
    in_proj_w = np.asarray(inputs["in_proj_w"], f32)
    conv_w = np.asarray(inputs["conv_w"], f32)
    conv_b = np.asarray(inputs["conv_b"], f32)
    x_proj_w = np.asarray(inputs["x_proj_w"], f32)
    dt_proj_w = np.asarray(inputs["dt_proj_w"], f32)
    dt_proj_b = np.asarray(inputs["dt_proj_b"], f32)
    A_log = np.asarray(inputs["A_log"], f32)
    Dvec = np.asarray(inputs["D"], f32)
    out_proj_w = np.asarray(inputs["out_proj_w"], f32)
    ln_w = np.asarray(inputs["ln_w"], f32)
    ln_b = np.asarray(inputs["ln_b"], f32)
    n2_w = np.asarray(inputs["norm2_w"], f32)
    n2_b = np.asarray(inputs["norm2_b"], f32)
    cw1 = np.asarray(inputs["cw1"], f32)
    cw2 = np.asarray(inputs["cw2"], f32)
    cb1 = np.asarray(inputs["cb1"], f32)
    cb2 = np.asarray(inputs["cb2"], f32)

    n = np.arange(L, dtype=np.float64)
    ang = 2.0 * np.pi * np.outer(n, n) / L
    Cdft = (np.cos(ang) / np.sqrt(L)).astype(BF16)
    Sdft = (np.sin(ang) / np.sqrt(L)).astype(BF16)
    CdF = [np.ascontiguousarray(Cdft[:, r * 256:(r + 1) * 256])
           for r in range(GROUP)]
    SdF = [np.ascontiguousarray(Sdft[:, r * 256:(r + 1) * 256])
           for r in range(GROUP)]
    CdI = [np.ascontiguousarray(Cdft[r * 256:(r + 1) * 256, :])
           for r in range(GROUP)]
    SdI = [np.ascontiguousarray(Sdft[r * 256:(r + 1) * 256, :])
           for r in range(GROUP)]

    ssb = np.stack([
        (cb2[0] - LAM) / 2, (-cb2[0] - LAM) / 2,
        (cb2[1] - LAM) / 2, (-cb2[1] - LAM) / 2,
    ], axis=1)[:, :, :, None]

    in_maps = []
    for core in range(N_CORES):
        g, r = divmod(core, GROUP)
        lo, hi = r * DIL, (r + 1) * DIL
        # per-core channel order: own d_inner slice first, then the rest
        perm = np.r_[lo:hi, 0:lo, hi:DI]
        conv_wp = conv_w[perm, 0, :]
        conv_diag = np.zeros((NMT * DC, P, P), f32)
        for mt in range(NMT):
            for q in range(DC):
                np.fill_diagonal(conv_diag[mt * DC + q],
                                 conv_wp[mt * P:(mt + 1) * P, q])
        diag_D = np.zeros((NDT, P, P), f32)
        for j in range(NDT):
            np.fill_diagonal(diag_D[j], Dvec[lo + j * P: lo + (j + 1) * P])
        m = {
            "xT": np.ascontiguousarray(x[g].T).astype(BF16),
            "w_in": np.ascontiguousarray(
                np.concatenate([in_proj_w[perm],
                                in_proj_w[DI + lo:DI + hi]], 0).T
            ).astype(BF16),
            "conv_diag": conv_diag.astype(BF16),
            "conv_b": np.ascontiguousarray(conv_b[perm][:, None]),
            "w_xp": np.ascontiguousarray(x_proj_w[:, perm].T).astype(BF16),
            "w_dt": np.ascontiguousarray(dt_proj_w[lo:hi].T).astype(BF16),
            "dt_b": np.ascontiguousarray(dt_proj_b[lo:hi][:, None]),
            "A": np.ascontiguousarray(-np.exp(A_log[lo:hi])),
            "diag_D": diag_D.astype(BF16),
            "w_out": np.ascontiguousarray(
                out_proj_w[:, lo:hi].T).astype(BF16),
            "ln_w": np.ascontiguousarray(ln_w[:, None]),
            "ln_b": np.ascontiguousarray(ln_b[:, None]),
            "n2_w": np.ascontiguousarray(n2_w[:, None]),
            "n2_b": np.ascontiguousarray(n2_b[:, None]),
            "CdF": CdF[r], "SdF": SdF[r],
            "CdI": CdI[r], "SdI": SdI[r],
            "w1r": (0.5 * cw1[0]).astype(BF16),
            "w1i": (0.5 * cw1[1]).astype(BF16),
            "w1in": (-0.5 * cw1[1]).astype(BF16),
            "w2r": cw2[0].astype(BF16),
            "w2i": cw2[1].astype(BF16),
            "w2in": (-cw2[1]).astype(BF16),
            "cb1r": np.ascontiguousarray(cb1[0][:, :, None]),
            "cb1i": np.ascontiguousarray(cb1[1][:, :, None]),
            "ssb": np.ascontiguousarray(ssb, f32),
            "ident": np.eye(P, dtype=f32),
        }
        in_maps.append(m)
    return in_maps


def kernel(**inputs):
    global _COMPILED
    from concourse.bass_utils import run_bass_kernel_spmd
    if _COMPILED is None:
        _COMPILED = _build_program()
    in_maps = _make_inmaps(inputs)
    res = run_bass_kernel_spmd(_COMPILED, in_maps,
                               core_ids=list(range(N_CORES)))
    outs = []
    for g in range(2):
        x = res.results[g * GROUP]["xO"].astype(np.float32)
        for r in range(GROUP):
            x = x + res.results[g * GROUP + r]["xP"].astype(np.float32)
        outs.append(x.T)
    return np.ascontiguousarray(np.stack(outs).astype(np.float32))
